# revision 18
# baseline (speedup 1.0000x reference)
"""Trainium2 Bass kernel for 2-layer single-head GAT (nn_GAT__80942953660642).

Strategy (8 NeuronCores, SPMD):
  - Nodes sharded contiguously: core c owns nodes [c*12500, (c+1)*12500).
  - Phase M: h = X_shard @ W0 on PE (host-pretransposed X tiles), el/er via
    DVE reduces; rows [h(140), el, er, pad] packed into a 144-f32 shard table.
  - AllGather the 7.2MB shard tables -> full 57.8MB node table per core.
  - Phase E0 (edge phase): per core, its dst nodes are degree-sorted into 98
    groups of 128 (one dst per partition). Each dst's incoming edges occupy
    padded slot columns; slot gathers use per-partition indirect DMA (128
    rows/call, int32 element offsets). Edge softmax without max-subtraction
    (numerically safe here); padding slots point at sentinel rows with
    el = -1e30 so exp() kills them. Weighted accumulation via fused DVE
    multiply-add over slot columns.
  - hp1 = h1 @ W1 (PE transpose + matmul), second 16-f32 table, AllGather,
    Phase E1 repeats the edge phase at width 7.
  - Host assembles the final [100000, 7] output (inverse degree-sort).
"""
import sys
sys.path.insert(0, "/opt/trn_rl_repo")
import numpy as np

N = 100000
NCORES = 8
SHARD = 12500
PSHARD = 12544          # 98 * 128
G = PSHARD // 128       # 98 groups
KDIM = 1536             # 1433 padded to 12*128
D0 = 140
D1 = 7
W0C = 144               # L0 table row: h(140), el(140), er(141), pad
W1C = 16                # L1 table row: hp1(7), el1(7), er1(8), pad
SENT = np.float32(-1e30)

_CACHE = {}


def _host_prep(src, dst):
    src = np.asarray(src).astype(np.int64)
    dst = np.asarray(dst).astype(np.int64)
    deg = np.bincount(dst, minlength=N)
    nodes = np.arange(N, dtype=np.int64)
    pad_id = (nodes // SHARD) * PSHARD + (nodes % SHARD)  # original-order padded id

    percore = []
    Kg = np.zeros(G, dtype=np.int64)
    for c in range(NCORES):
        lo = c * SHARD
        m = (dst >= lo) & (dst < lo + SHARD)
        e_dst = dst[m] - lo
        e_src = src[m]
        d = deg[lo:lo + SHARD]
        order = np.argsort(-d, kind="stable")
        rank = np.empty(SHARD, dtype=np.int64)
        rank[order] = np.arange(SHARD)
        dsort = np.concatenate([d[order], np.zeros(PSHARD - SHARD, np.int64)])
        for g in range(G):
            Kg[g] = max(Kg[g], max(1, dsort[g * 128:(g + 1) * 128].max()))
        percore.append(dict(order=order, rank=rank, e_dst=e_dst, e_src=e_src))

    pos1 = np.empty(N, dtype=np.int64)  # sorted-space padded id
    for c in range(NCORES):
        pos1[c * SHARD:(c + 1) * SHARD] = c * PSHARD + percore[c]["rank"]

    SK = int(Kg.sum())
    cums = np.concatenate([[0], np.cumsum(Kg)])
    for c in range(NCORES):
        pc = percore[c]
        sent_row = c * PSHARD + PSHARD - 1
        r = pc["rank"][pc["e_dst"]]
        ordr = np.argsort(r, kind="stable")
        r_s = r[ordr]
        kpos = np.arange(len(r_s)) - np.searchsorted(r_s, r_s)
        Kcap = int(Kg.max())
        slots0 = np.full((PSHARD, Kcap), sent_row, dtype=np.int64)
        slots0[r_s, kpos] = pad_id[pc["e_src"][ordr]]
        slots1 = np.full((PSHARD, Kcap), sent_row, dtype=np.int64)
        slots1[r_s, kpos] = pos1[pc["e_src"][ordr]]
        # pack per-group [128, Kg[g]] -> [128, SK] (element offsets)
        offs0 = np.zeros((128, SK), np.int32)
        offs1 = np.zeros((128, SK), np.int32)
        for g in range(G):
            offs0[:, cums[g]:cums[g + 1]] = slots0[g * 128:(g + 1) * 128, :Kg[g]]
            offs1[:, cums[g]:cums[g + 1]] = slots1[g * 128:(g + 1) * 128, :Kg[g]]
        pc["offs0"] = offs0
        pc["offs1"] = offs1
        # dst-row er element offsets per group [128, G]
        dr = np.empty((128, G), np.int32)
        for g in range(G):
            if (g + 1) * 128 <= SHARD:
                ids = pc["order"][g * 128:(g + 1) * 128]
            else:
                ids = np.concatenate([pc["order"][g * 128:SHARD],
                                      np.full((g + 1) * 128 - SHARD, SHARD, np.int64)])
                ids[SHARD - g * 128:] = PSHARD - 1  # ghosts -> own pad row
            dr[:, g] = c * PSHARD + ids
        pc["dst_er"] = dr
    return percore, Kg.astype(int), cums.astype(int)


def _prep_weights(inputs):
    X = np.asarray(inputs["X"], np.float32)
    W0 = np.asarray(inputs["W0"], np.float32)
    al0 = np.asarray(inputs["al0"], np.float32)
    ar0 = np.asarray(inputs["ar0"], np.float32)
    b0 = np.asarray(inputs["b0"], np.float32)
    W1 = np.asarray(inputs["W1"], np.float32)
    al1 = np.asarray(inputs["al1"], np.float32)
    ar1 = np.asarray(inputs["ar1"], np.float32)
    b1 = np.asarray(inputs["b1"], np.float32)

    # W0 rearranged: [128, 12*140]; W0r[kp, k*140+j] = W0[k*128+kp, j]
    W0p = np.zeros((KDIM, D0), np.float32)
    W0p[:1433] = W0
    W0r = np.ascontiguousarray(
        W0p.reshape(12, 128, D0).transpose(1, 0, 2).reshape(128, 12 * D0))
    W1a = np.zeros((128, D1), np.float32)
    W1a[:128] = W1[:128]
    W1b = np.zeros((128, D1), np.float32)
    W1b[:12] = W1[128:140]
    bc = lambda v, w: np.broadcast_to(np.asarray(v, np.float32)[None, :],
                                      (128, w)).copy()
    wal1 = W1 @ al1
    war1 = W1 @ ar1
    ident = np.eye(128, dtype=np.float32)
    sent_mask = np.zeros((128, 1), np.float32)
    sent_mask[SHARD - (G - 1) * 128:, 0] = SENT   # partitions 84.. are pads
    com = dict(W0r=W0r, W1a=W1a, W1b=W1b, sent_mask=sent_mask,
               al0b=bc(al0, D0), ar0b=bc(ar0, D0), b0b=bc(b0, D0),
               wal1b=bc(wal1, D0), war1b=bc(war1, D0), b1b=bc(b1, D1),
               ident=ident)

    # X tiles per core: xt[n, kp, k*128+nf] = X[lo + n*128+nf, k*128+kp]
    xts = []
    for c in range(NCORES):
        lo = c * SHARD
        Xp = np.zeros((PSHARD, KDIM), np.float32)
        Xp[:SHARD, :1433] = X[lo:lo + SHARD, :]
        xt = np.ascontiguousarray(
            Xp.reshape(G, 128, 12, 128).transpose(0, 3, 2, 1).reshape(G, 128, 12 * 128))
        xts.append(xt)
    return com, xts


def _build(Kg, cums):
    import concourse.bass as bass
    import concourse.tile as tile
    from concourse import bacc, mybir
    dt = mybir.dt
    op = mybir.AluOpType
    act = mybir.ActivationFunctionType

    SK = int(sum(Kg))
    nc = bacc.Bacc("TRN2", target_bir_lowering=False, debug=False,
                   num_devices=NCORES)
    t_x = nc.dram_tensor("x_up", [G, 128, 12 * 128], dt.float32, kind="ExternalInput")
    t_w0 = nc.dram_tensor("w0r", [128, 12 * D0], dt.float32, kind="ExternalInput")
    t_w1a = nc.dram_tensor("w1a", [128, D1], dt.float32, kind="ExternalInput")
    t_w1b = nc.dram_tensor("w1b", [128, D1], dt.float32, kind="ExternalInput")
    t_al0 = nc.dram_tensor("al0b", [128, D0], dt.float32, kind="ExternalInput")
    t_ar0 = nc.dram_tensor("ar0b", [128, D0], dt.float32, kind="ExternalInput")
    t_b0 = nc.dram_tensor("b0b", [128, D0], dt.float32, kind="ExternalInput")
    t_wal1 = nc.dram_tensor("wal1b", [128, D0], dt.float32, kind="ExternalInput")
    t_war1 = nc.dram_tensor("war1b", [128, D0], dt.float32, kind="ExternalInput")
    t_b1 = nc.dram_tensor("b1b", [128, D1], dt.float32, kind="ExternalInput")
    t_id = nc.dram_tensor("ident", [128, 128], dt.float32, kind="ExternalInput")
    t_of0 = nc.dram_tensor("offs0", [128, SK], dt.int32, kind="ExternalInput")
    t_of1 = nc.dram_tensor("offs1", [128, SK], dt.int32, kind="ExternalInput")
    t_der = nc.dram_tensor("dst_er", [128, G], dt.int32, kind="ExternalInput")
    t_sm = nc.dram_tensor("sent_mask", [128, 1], dt.float32, kind="ExternalInput")
    t_out = nc.dram_tensor("out_buf", [PSHARD, D1], dt.float32, kind="ExternalOutput")

    with tile.TileContext(nc) as tc:
        with tc.tile_pool(name="const", bufs=1) as cpool, \
             tc.tile_pool(name="xload", bufs=2) as xpool, \
             tc.tile_pool(name="hex", bufs=3) as hexpool, \
             tc.tile_pool(name="gath", bufs=2) as gpool, \
             tc.tile_pool(name="work", bufs=3) as wpool, \
             tc.tile_pool(name="small", bufs=4) as spool, \
             tc.tile_pool(name="psum", bufs=2, space="PSUM") as ppool, \
             tc.tile_pool(name="dram", bufs=1, space="DRAM") as dpool:

            # constants
            w0_sb = cpool.tile([128, 12 * D0], dt.float32)
            nc.sync.dma_start(w0_sb[:], t_w0[:])
            w1a_sb = cpool.tile([128, D1], dt.float32)
            nc.sync.dma_start(w1a_sb[:], t_w1a[:])
            w1b_sb = cpool.tile([128, D1], dt.float32)
            nc.sync.dma_start(w1b_sb[:], t_w1b[:])
            al0_sb = cpool.tile([128, D0], dt.float32)
            nc.sync.dma_start(al0_sb[:], t_al0[:])
            ar0_sb = cpool.tile([128, D0], dt.float32)
            nc.sync.dma_start(ar0_sb[:], t_ar0[:])
            b0_sb = cpool.tile([128, D0], dt.float32)
            nc.sync.dma_start(b0_sb[:], t_b0[:])
            wal1_sb = cpool.tile([128, D0], dt.float32)
            nc.sync.dma_start(wal1_sb[:], t_wal1[:])
            war1_sb = cpool.tile([128, D0], dt.float32)
            nc.sync.dma_start(war1_sb[:], t_war1[:])
            b1_sb = cpool.tile([128, D1], dt.float32)
            nc.sync.dma_start(b1_sb[:], t_b1[:])
            id_sb = cpool.tile([128, 128], dt.float32)
            nc.sync.dma_start(id_sb[:], t_id[:])
            of0_sb = cpool.tile([128, SK], dt.int32)
            nc.sync.dma_start(of0_sb[:], t_of0[:])
            of1_sb = cpool.tile([128, SK], dt.int32)
            nc.sync.dma_start(of1_sb[:], t_of1[:])
            der_sb = cpool.tile([128, G], dt.int32)
            nc.sync.dma_start(der_sb[:], t_der[:])
            sm_sb = cpool.tile([128, 1], dt.float32)
            nc.sync.dma_start(sm_sb[:], t_sm[:])
            er1_all = cpool.tile([128, G], dt.float32)

            shard0 = dpool.tile([PSHARD, W0C], dt.float32)
            table0 = dpool.tile([NCORES * PSHARD, W0C], dt.float32, addr_space="Shared")
            shard1 = dpool.tile([PSHARD, W1C], dt.float32)
            table1 = dpool.tile([NCORES * PSHARD, W1C], dt.float32, addr_space="Shared")

            # ---- Phase M: h = X @ W0, pack [h, el, er] rows ----
            for n in range(G):
                xt = xpool.tile([128, 12 * 128], dt.float32)
                nc.sync.dma_start(xt[:], t_x[:][n])
                ph = ppool.tile([128, D0], dt.float32, space="PSUM")
                for k in range(12):
                    nc.tensor.matmul(ph[:], xt[:, k * 128:(k + 1) * 128],
                                     w0_sb[:, k * D0:(k + 1) * D0],
                                     start=(k == 0), stop=(k == 11))
                hx = hexpool.tile([128, W0C], dt.float32, tag="hex0")
                nc.vector.tensor_copy(hx[:, 0:D0], ph[:])
                scr = wpool.tile([128, D0], dt.float32, tag="mscr")
                nc.vector.tensor_tensor(scr[:], ph[:], al0_sb[:], op=op.mult)
                nc.vector.tensor_reduce(hx[:, 140:141], scr[:],
                                        axis=mybir.AxisListType.X, op=op.add)
                nc.vector.tensor_tensor(scr[:], ph[:], ar0_sb[:], op=op.mult)
                nc.vector.tensor_reduce(hx[:, 141:142], scr[:],
                                        axis=mybir.AxisListType.X, op=op.add)
                nc.vector.memset(hx[:, 142:144], 0.0)
                if n == G - 1:
                    nc.vector.tensor_tensor(hx[:, 140:141], hx[:, 140:141],
                                            sm_sb[:], op=op.add)
                nc.sync.dma_start(shard0[:].rearrange("(g p) w -> g p w", p=128)[n],
                                  hx[:])

            nc.gpsimd.collective_compute(
                "AllGather", op.bypass, replica_groups=[list(range(NCORES))],
                ins=[shard0[:]], outs=[table0[:]])

            # ---- Phase E0 ----
            for g in range(G):
                K = int(Kg[g])
                er_t = spool.tile([128, 1], dt.float32, tag="er0")
                nc.gpsimd.indirect_dma_start(
                    out=er_t[:], out_offset=None, in_=table0[:],
                    in_offset=bass.IndirectOffsetOnAxis(ap=der_sb[:, g:g + 1], axis=0),
                    element_offset=141)
                gt = gpool.tile([128, K * W0C], dt.float32, tag="g0")
                gv = gt[:].rearrange("p (k w) -> p k w", w=W0C)
                for k in range(K):
                    nc.gpsimd.indirect_dma_start(
                        out=gv[:, k], out_offset=None, in_=table0[:],
                        in_offset=bass.IndirectOffsetOnAxis(
                            ap=of0_sb[:, cums[g] + k:cums[g] + k + 1], axis=0))
                ep = spool.tile([128, K], dt.float32, tag="ep0")
                nc.vector.tensor_scalar(ep[:], gv[:, :, 140], er_t[:], None, op.add)
                ee = spool.tile([128, K], dt.float32, tag="ee0")
                nc.vector.scalar_tensor_tensor(
                    out=ee[:], in0=ep[:], scalar=0.2, in1=ep[:],
                    op0=op.mult, op1=op.max)
                ex = spool.tile([128, K], dt.float32, tag="ex0")
                dn = spool.tile([128, 1], dt.float32, tag="dn0")
                nc.scalar.activation(ex[:], ee[:], act.Exp, accum_out=dn[:])
                nc.vector.tensor_scalar_max(dn[:], dn[:], 1e-30)
                rv = spool.tile([128, 1], dt.float32, tag="rv0")
                nc.vector.reciprocal(rv[:], dn[:])
                acc = wpool.tile([128, D0], dt.float32, tag="acc0")
                nc.vector.tensor_scalar(acc[:], gv[:, 0, 0:D0], ex[:, 0:1], None,
                                        op.mult)
                for k in range(1, K):
                    nc.vector.scalar_tensor_tensor(
                        out=acc[:], in0=gv[:, k, 0:D0], scalar=ex[:, k:k + 1],
                        in1=acc[:], op0=op.mult, op1=op.add)
                h1 = wpool.tile([128, D0], dt.float32, tag="h1")
                nc.vector.scalar_tensor_tensor(
                    out=h1[:], in0=acc[:], scalar=rv[:], in1=b0_sb[:],
                    op0=op.mult, op1=op.add)
                nc.scalar.activation(h1[:], h1[:], act.Relu)
                # el1/er1
                hx1 = hexpool.tile([128, W1C], dt.float32, tag="hex1")
                scr1 = wpool.tile([128, D0], dt.float32, tag="escr")
                nc.vector.tensor_tensor(scr1[:], h1[:], wal1_sb[:], op=op.mult)
                nc.vector.tensor_reduce(hx1[:, 7:8], scr1[:],
                                        axis=mybir.AxisListType.X, op=op.add)
                nc.vector.tensor_tensor(scr1[:], h1[:], war1_sb[:], op=op.mult)
                nc.vector.tensor_reduce(hx1[:, 8:9], scr1[:],
                                        axis=mybir.AxisListType.X, op=op.add)
                nc.vector.tensor_copy(er1_all[:, g:g + 1], hx1[:, 8:9])
                # hp1 = h1 @ W1 via PE transpose
                pt1 = ppool.tile([128, 128], dt.float32, space="PSUM", tag="pt1")
                nc.tensor.transpose(pt1[:], h1[:, 0:128], id_sb[:])
                pt2 = ppool.tile([128, 128], dt.float32, space="PSUM", tag="pt2")
                nc.tensor.transpose(pt2[0:12, :], h1[:, 128:140], id_sb[:])
                t1s = wpool.tile([128, 128], dt.float32, tag="t1s")
                nc.vector.tensor_copy(t1s[:], pt1[:])
                t2s = wpool.tile([128, 128], dt.float32, tag="t2s")
                nc.vector.tensor_copy(t2s[0:12, :], pt2[0:12, :])
                php = ppool.tile([128, D1], dt.float32, space="PSUM", tag="php")
                nc.tensor.matmul(php[:], t1s[:], w1a_sb[:], start=True, stop=False)
                nc.tensor.matmul(php[:], t2s[0:12, :], w1b_sb[0:12, :],
                                 start=False, stop=True)
                nc.vector.tensor_copy(hx1[:, 0:D1], php[:])
                nc.vector.memset(hx1[:, 9:16], 0.0)
                if g == G - 1:
                    nc.vector.tensor_tensor(hx1[:, 7:8], hx1[:, 7:8],
                                            sm_sb[:], op=op.add)
                nc.sync.dma_start(shard1[:].rearrange("(g p) w -> g p w", p=128)[g],
                                  hx1[:])

            nc.gpsimd.collective_compute(
                "AllGather", op.bypass, replica_groups=[list(range(NCORES))],
                ins=[shard1[:]], outs=[table1[:]])

            # ---- Phase E1 ----
            for g in range(G):
                K = int(Kg[g])
                gt = gpool.tile([128, K * W1C], dt.float32, tag="g1")
                gv = gt[:].rearrange("p (k w) -> p k w", w=W1C)
                for k in range(K):
                    nc.gpsimd.indirect_dma_start(
                        out=gv[:, k], out_offset=None, in_=table1[:],
                        in_offset=bass.IndirectOffsetOnAxis(
                            ap=of1_sb[:, cums[g] + k:cums[g] + k + 1], axis=0))
                ep = spool.tile([128, K], dt.float32, tag="ep1")
                nc.vector.tensor_scalar(ep[:], gv[:, :, 7], er1_all[:, g:g + 1],
                                        None, op.add)
                ee = spool.tile([128, K], dt.float32, tag="ee1")
                nc.vector.scalar_tensor_tensor(
                    out=ee[:], in0=ep[:], scalar=0.2, in1=ep[:],
                    op0=op.mult, op1=op.max)
                ex = spool.tile([128, K], dt.float32, tag="ex1")
                dn = spool.tile([128, 1], dt.float32, tag="dn1")
                nc.scalar.activation(ex[:], ee[:], act.Exp, accum_out=dn[:])
                nc.vector.tensor_scalar_max(dn[:], dn[:], 1e-30)
                rv = spool.tile([128, 1], dt.float32, tag="rv1")
                nc.vector.reciprocal(rv[:], dn[:])
                acc = spool.tile([128, D1], dt.float32, tag="acc1")
                nc.vector.tensor_scalar(acc[:], gv[:, 0, 0:D1], ex[:, 0:1], None,
                                        op.mult)
                for k in range(1, K):
                    nc.vector.scalar_tensor_tensor(
                        out=acc[:], in0=gv[:, k, 0:D1], scalar=ex[:, k:k + 1],
                        in1=acc[:], op0=op.mult, op1=op.add)
                ot = spool.tile([128, D1], dt.float32, tag="ot")
                nc.vector.scalar_tensor_tensor(
                    out=ot[:], in0=acc[:], scalar=rv[:], in1=b1_sb[:],
                    op0=op.mult, op1=op.add)
                nc.scalar.activation(ot[:], ot[:], act.Relu)
                nc.sync.dma_start(t_out[:].rearrange("(g p) w -> g p w", p=128)[g],
                                  ot[:])
    nc.compile()
    return nc


def kernel(**inputs):
    percore, Kg, cums = _host_prep(inputs["src"], inputs["dst"])
    com, xts = _prep_weights(inputs)

    key = tuple(Kg)
    if key not in _CACHE:
        _CACHE[key] = _build(Kg, cums)
    nc = _CACHE[key]

    in_maps = []
    for c in range(NCORES):
        pc = percore[c]
        m = dict(x_up=xts[c], w0r=com["W0r"], w1a=com["W1a"], w1b=com["W1b"],
                 al0b=com["al0b"], ar0b=com["ar0b"], b0b=com["b0b"],
                 wal1b=com["wal1b"], war1b=com["war1b"], b1b=com["b1b"],
                 ident=com["ident"], offs0=pc["offs0"], offs1=pc["offs1"],
                 dst_er=pc["dst_er"], sent_mask=com["sent_mask"])
        in_maps.append(m)

    from concourse.bass_utils import run_bass_kernel_spmd
    res = run_bass_kernel_spmd(nc, in_maps, core_ids=list(range(NCORES)),
                               trace=False)
    out = np.zeros((N, D1), dtype=np.float32)
    for c in range(NCORES):
        ob = res.results[c]["out_buf"]
        out[c * SHARD + percore[c]["order"]] = ob[:SHARD]
    return out


# revision 20
# speedup vs baseline: 2142.6464x; 2142.6464x over previous
"""Trainium2 Bass kernel for 2-layer single-head GAT (nn_GAT__80942953660642).

Strategy (8 NeuronCores, SPMD):
  - Nodes sharded contiguously: core c owns nodes [c*12500, (c+1)*12500).
  - Phase M: h = X_shard @ W0 on PE (host-pretransposed X tiles), el/er via
    DVE reduces; rows [h(140), el, er, pad] packed into a 144-f32 shard table.
  - AllGather the 7.2MB shard tables -> full 57.8MB node table per core.
  - Phase E0 (edge phase): per core, its dst nodes are degree-sorted into 98
    groups of 128 (one dst per partition). Each dst's incoming edges occupy
    padded slot columns; slot gathers use per-partition indirect DMA (128
    rows/call, int32 element offsets). Edge softmax without max-subtraction
    (numerically safe here); padding slots point at sentinel rows with
    el = -1e30 so exp() kills them. Weighted accumulation via fused DVE
    multiply-add over slot columns.
  - hp1 = h1 @ W1 (PE transpose + matmul), second 16-f32 table, AllGather,
    Phase E1 repeats the edge phase at width 7.
  - Host assembles the final [100000, 7] output (inverse degree-sort).
"""
import sys
sys.path.insert(0, "/opt/trn_rl_repo")
import numpy as np

N = 100000
NCORES = 8
SHARD = 12500
PSHARD = 12544          # 98 * 128
G = PSHARD // 128       # 98 groups
KDIM = 1536             # 1433 padded to 12*128
D0 = 140
D1 = 7
W0C = 144               # L0 table row: h(140), el(140), er(141), pad
W1C = 16                # L1 table row: hp1(7), el1(7), er1(8), pad
SENT = np.float32(-1e30)

_CACHE = {}
TRACE = False          # test harness sets this to capture an NTFF profile
LAST_EXEC_NS = None


def _host_prep(src, dst):
    src = np.asarray(src).astype(np.int64)
    dst = np.asarray(dst).astype(np.int64)
    deg = np.bincount(dst, minlength=N)
    nodes = np.arange(N, dtype=np.int64)
    pad_id = (nodes // SHARD) * PSHARD + (nodes % SHARD)  # original-order padded id

    percore = []
    Kg = np.zeros(G, dtype=np.int64)
    for c in range(NCORES):
        lo = c * SHARD
        m = (dst >= lo) & (dst < lo + SHARD)
        e_dst = dst[m] - lo
        e_src = src[m]
        d = deg[lo:lo + SHARD]
        order = np.argsort(-d, kind="stable")
        rank = np.empty(SHARD, dtype=np.int64)
        rank[order] = np.arange(SHARD)
        dsort = np.concatenate([d[order], np.zeros(PSHARD - SHARD, np.int64)])
        for g in range(G):
            Kg[g] = max(Kg[g], max(1, dsort[g * 128:(g + 1) * 128].max()))
        percore.append(dict(order=order, rank=rank, e_dst=e_dst, e_src=e_src))

    pos1 = np.empty(N, dtype=np.int64)  # sorted-space padded id
    for c in range(NCORES):
        pos1[c * SHARD:(c + 1) * SHARD] = c * PSHARD + percore[c]["rank"]

    SK = int(Kg.sum())
    cums = np.concatenate([[0], np.cumsum(Kg)])
    for c in range(NCORES):
        pc = percore[c]
        sent_row = c * PSHARD + PSHARD - 1
        r = pc["rank"][pc["e_dst"]]
        ordr = np.argsort(r, kind="stable")
        r_s = r[ordr]
        kpos = np.arange(len(r_s)) - np.searchsorted(r_s, r_s)
        Kcap = int(Kg.max())
        slots0 = np.full((PSHARD, Kcap), sent_row, dtype=np.int64)
        slots0[r_s, kpos] = pad_id[pc["e_src"][ordr]]
        slots1 = np.full((PSHARD, Kcap), sent_row, dtype=np.int64)
        slots1[r_s, kpos] = pos1[pc["e_src"][ordr]]
        # pack per-group [128, Kg[g]] -> [128, SK] (element offsets)
        offs0 = np.zeros((128, SK), np.int32)
        offs1 = np.zeros((128, SK), np.int32)
        for g in range(G):
            offs0[:, cums[g]:cums[g + 1]] = slots0[g * 128:(g + 1) * 128, :Kg[g]]
            offs1[:, cums[g]:cums[g + 1]] = slots1[g * 128:(g + 1) * 128, :Kg[g]]
        pc["offs0"] = offs0
        pc["offs1"] = offs1
        # dst-row er element offsets per group [128, G]
        dr = np.empty((128, G), np.int32)
        for g in range(G):
            if (g + 1) * 128 <= SHARD:
                ids = pc["order"][g * 128:(g + 1) * 128]
            else:
                ids = np.concatenate([pc["order"][g * 128:SHARD],
                                      np.full((g + 1) * 128 - SHARD, SHARD, np.int64)])
                ids[SHARD - g * 128:] = PSHARD - 1  # ghosts -> own pad row
            dr[:, g] = c * PSHARD + ids
        pc["dst_er"] = dr
    return percore, Kg.astype(int), cums.astype(int)


def _prep_weights(inputs):
    X = np.asarray(inputs["X"], np.float32)
    W0 = np.asarray(inputs["W0"], np.float32)
    al0 = np.asarray(inputs["al0"], np.float32)
    ar0 = np.asarray(inputs["ar0"], np.float32)
    b0 = np.asarray(inputs["b0"], np.float32)
    W1 = np.asarray(inputs["W1"], np.float32)
    al1 = np.asarray(inputs["al1"], np.float32)
    ar1 = np.asarray(inputs["ar1"], np.float32)
    b1 = np.asarray(inputs["b1"], np.float32)

    # W0 rearranged: [128, 12*140]; W0r[kp, k*140+j] = W0[k*128+kp, j]
    W0p = np.zeros((KDIM, D0), np.float32)
    W0p[:1433] = W0
    W0r = np.ascontiguousarray(
        W0p.reshape(12, 128, D0).transpose(1, 0, 2).reshape(128, 12 * D0))
    W1a = np.zeros((128, D1), np.float32)
    W1a[:128] = W1[:128]
    W1b = np.zeros((128, D1), np.float32)
    W1b[:12] = W1[128:140]
    bc = lambda v, w: np.broadcast_to(np.asarray(v, np.float32)[None, :],
                                      (128, w)).copy()
    wal1 = W1 @ al1
    war1 = W1 @ ar1
    ident = np.eye(128, dtype=np.float32)
    sent_mask = np.zeros((128, 1), np.float32)
    sent_mask[SHARD - (G - 1) * 128:, 0] = SENT   # partitions 84.. are pads
    com = dict(W0r=W0r, W1a=W1a, W1b=W1b, sent_mask=sent_mask,
               al0b=bc(al0, D0), ar0b=bc(ar0, D0), b0b=bc(b0, D0),
               wal1b=bc(wal1, D0), war1b=bc(war1, D0), b1b=bc(b1, D1),
               ident=ident)

    # X tiles per core: xt[n, kp, k*128+nf] = X[lo + n*128+nf, k*128+kp]
    xts = []
    for c in range(NCORES):
        lo = c * SHARD
        Xp = np.zeros((PSHARD, KDIM), np.float32)
        Xp[:SHARD, :1433] = X[lo:lo + SHARD, :]
        xt = np.ascontiguousarray(
            Xp.reshape(G, 128, 12, 128).transpose(0, 3, 2, 1).reshape(G, 128, 12 * 128))
        xts.append(xt)
    return com, xts


def _build(Kg, cums):
    import concourse.bass as bass
    import concourse.tile as tile
    from concourse import bacc, mybir
    dt = mybir.dt
    op = mybir.AluOpType
    act = mybir.ActivationFunctionType

    SK = int(sum(Kg))
    nc = bacc.Bacc("TRN2", target_bir_lowering=False, debug=False,
                   num_devices=NCORES)
    t_x = nc.dram_tensor("x_up", [G, 128, 12 * 128], dt.float32, kind="ExternalInput")
    t_w0 = nc.dram_tensor("w0r", [128, 12 * D0], dt.float32, kind="ExternalInput")
    t_w1a = nc.dram_tensor("w1a", [128, D1], dt.float32, kind="ExternalInput")
    t_w1b = nc.dram_tensor("w1b", [128, D1], dt.float32, kind="ExternalInput")
    t_al0 = nc.dram_tensor("al0b", [128, D0], dt.float32, kind="ExternalInput")
    t_ar0 = nc.dram_tensor("ar0b", [128, D0], dt.float32, kind="ExternalInput")
    t_b0 = nc.dram_tensor("b0b", [128, D0], dt.float32, kind="ExternalInput")
    t_wal1 = nc.dram_tensor("wal1b", [128, D0], dt.float32, kind="ExternalInput")
    t_war1 = nc.dram_tensor("war1b", [128, D0], dt.float32, kind="ExternalInput")
    t_b1 = nc.dram_tensor("b1b", [128, D1], dt.float32, kind="ExternalInput")
    t_id = nc.dram_tensor("ident", [128, 128], dt.float32, kind="ExternalInput")
    t_of0 = nc.dram_tensor("offs0", [128, SK], dt.int32, kind="ExternalInput")
    t_of1 = nc.dram_tensor("offs1", [128, SK], dt.int32, kind="ExternalInput")
    t_der = nc.dram_tensor("dst_er", [128, G], dt.int32, kind="ExternalInput")
    t_sm = nc.dram_tensor("sent_mask", [128, 1], dt.float32, kind="ExternalInput")
    t_out = nc.dram_tensor("out_buf", [PSHARD, D1], dt.float32, kind="ExternalOutput")

    with tile.TileContext(nc) as tc:
        with tc.tile_pool(name="const", bufs=1) as cpool, \
             tc.tile_pool(name="xload", bufs=2) as xpool, \
             tc.tile_pool(name="hex", bufs=3) as hexpool, \
             tc.tile_pool(name="gath", bufs=2) as gpool, \
             tc.tile_pool(name="work", bufs=3) as wpool, \
             tc.tile_pool(name="small", bufs=4) as spool, \
             tc.tile_pool(name="psum", bufs=2, space="PSUM") as ppool, \
             tc.tile_pool(name="dram", bufs=1, space="DRAM") as dpool:

            # constants
            w0_sb = cpool.tile([128, 12 * D0], dt.float32)
            nc.sync.dma_start(w0_sb[:], t_w0[:])
            w1a_sb = cpool.tile([128, D1], dt.float32)
            nc.sync.dma_start(w1a_sb[:], t_w1a[:])
            w1b_sb = cpool.tile([128, D1], dt.float32)
            nc.sync.dma_start(w1b_sb[:], t_w1b[:])
            al0_sb = cpool.tile([128, D0], dt.float32)
            nc.sync.dma_start(al0_sb[:], t_al0[:])
            ar0_sb = cpool.tile([128, D0], dt.float32)
            nc.sync.dma_start(ar0_sb[:], t_ar0[:])
            b0_sb = cpool.tile([128, D0], dt.float32)
            nc.sync.dma_start(b0_sb[:], t_b0[:])
            wal1_sb = cpool.tile([128, D0], dt.float32)
            nc.sync.dma_start(wal1_sb[:], t_wal1[:])
            war1_sb = cpool.tile([128, D0], dt.float32)
            nc.sync.dma_start(war1_sb[:], t_war1[:])
            b1_sb = cpool.tile([128, D1], dt.float32)
            nc.sync.dma_start(b1_sb[:], t_b1[:])
            id_sb = cpool.tile([128, 128], dt.float32)
            nc.sync.dma_start(id_sb[:], t_id[:])
            of0_sb = cpool.tile([128, SK], dt.int32)
            nc.sync.dma_start(of0_sb[:], t_of0[:])
            of1_sb = cpool.tile([128, SK], dt.int32)
            nc.sync.dma_start(of1_sb[:], t_of1[:])
            der_sb = cpool.tile([128, G], dt.int32)
            nc.sync.dma_start(der_sb[:], t_der[:])
            sm_sb = cpool.tile([128, 1], dt.float32)
            nc.sync.dma_start(sm_sb[:], t_sm[:])
            er1_all = cpool.tile([128, G], dt.float32)

            shard0 = dpool.tile([PSHARD, W0C], dt.float32)
            table0 = dpool.tile([NCORES * PSHARD, W0C], dt.float32, addr_space="Shared")
            shard1 = dpool.tile([PSHARD, W1C], dt.float32)
            table1 = dpool.tile([NCORES * PSHARD, W1C], dt.float32, addr_space="Shared")

            # ---- Phase M: h = X @ W0, pack [h, el, er] rows ----
            for n in range(G):
                xt = xpool.tile([128, 12 * 128], dt.float32)
                nc.sync.dma_start(xt[:], t_x[:][n])
                ph = ppool.tile([128, D0], dt.float32, space="PSUM")
                for k in range(12):
                    nc.tensor.matmul(ph[:], xt[:, k * 128:(k + 1) * 128],
                                     w0_sb[:, k * D0:(k + 1) * D0],
                                     start=(k == 0), stop=(k == 11))
                hx = hexpool.tile([128, W0C], dt.float32, tag="hex0")
                nc.vector.tensor_copy(hx[:, 0:D0], ph[:])
                scr = wpool.tile([128, D0], dt.float32, tag="mscr")
                nc.vector.tensor_tensor(scr[:], ph[:], al0_sb[:], op=op.mult)
                nc.vector.tensor_reduce(hx[:, 140:141], scr[:],
                                        axis=mybir.AxisListType.X, op=op.add)
                nc.vector.tensor_tensor(scr[:], ph[:], ar0_sb[:], op=op.mult)
                nc.vector.tensor_reduce(hx[:, 141:142], scr[:],
                                        axis=mybir.AxisListType.X, op=op.add)
                nc.vector.memset(hx[:, 142:144], 0.0)
                if n == G - 1:
                    nc.vector.tensor_tensor(hx[:, 140:141], hx[:, 140:141],
                                            sm_sb[:], op=op.add)
                nc.sync.dma_start(shard0[:].rearrange("(g p) w -> g p w", p=128)[n],
                                  hx[:])

            nc.gpsimd.collective_compute(
                "AllGather", op.bypass, replica_groups=[list(range(NCORES))],
                ins=[shard0[:]], outs=[table0[:]])

            # ---- Phase E0 ----
            for g in range(G):
                K = int(Kg[g])
                er_t = spool.tile([128, 1], dt.float32, tag="er0")
                nc.gpsimd.indirect_dma_start(
                    out=er_t[:], out_offset=None, in_=table0[:],
                    in_offset=bass.IndirectOffsetOnAxis(ap=der_sb[:, g:g + 1], axis=0),
                    element_offset=141)
                gt = gpool.tile([128, K * W0C], dt.float32, tag="g0")
                gv = gt[:].rearrange("p (k w) -> p k w", w=W0C)
                for k in range(K):
                    nc.gpsimd.indirect_dma_start(
                        out=gv[:, k], out_offset=None, in_=table0[:],
                        in_offset=bass.IndirectOffsetOnAxis(
                            ap=of0_sb[:, cums[g] + k:cums[g] + k + 1], axis=0))
                ep = spool.tile([128, K], dt.float32, tag="ep0")
                nc.vector.tensor_scalar(ep[:], gv[:, :, 140], er_t[:], None, op.add)
                ee = spool.tile([128, K], dt.float32, tag="ee0")
                nc.vector.scalar_tensor_tensor(
                    out=ee[:], in0=ep[:], scalar=0.2, in1=ep[:],
                    op0=op.mult, op1=op.max)
                ex = spool.tile([128, K], dt.float32, tag="ex0")
                dn = spool.tile([128, 1], dt.float32, tag="dn0")
                nc.scalar.activation(ex[:], ee[:], act.Exp, accum_out=dn[:])
                nc.vector.tensor_scalar_max(dn[:], dn[:], 1e-30)
                rv = spool.tile([128, 1], dt.float32, tag="rv0")
                nc.vector.reciprocal(rv[:], dn[:])
                acc = wpool.tile([128, D0], dt.float32, tag="acc0")
                nc.vector.tensor_scalar(acc[:], gv[:, 0, 0:D0], ex[:, 0:1], None,
                                        op.mult)
                for k in range(1, K):
                    nc.vector.scalar_tensor_tensor(
                        out=acc[:], in0=gv[:, k, 0:D0], scalar=ex[:, k:k + 1],
                        in1=acc[:], op0=op.mult, op1=op.add)
                h1 = wpool.tile([128, D0], dt.float32, tag="h1")
                nc.vector.scalar_tensor_tensor(
                    out=h1[:], in0=acc[:], scalar=rv[:], in1=b0_sb[:],
                    op0=op.mult, op1=op.add)
                nc.scalar.activation(h1[:], h1[:], act.Relu)
                # el1/er1
                hx1 = hexpool.tile([128, W1C], dt.float32, tag="hex1")
                scr1 = wpool.tile([128, D0], dt.float32, tag="escr")
                nc.vector.tensor_tensor(scr1[:], h1[:], wal1_sb[:], op=op.mult)
                nc.vector.tensor_reduce(hx1[:, 7:8], scr1[:],
                                        axis=mybir.AxisListType.X, op=op.add)
                nc.vector.tensor_tensor(scr1[:], h1[:], war1_sb[:], op=op.mult)
                nc.vector.tensor_reduce(hx1[:, 8:9], scr1[:],
                                        axis=mybir.AxisListType.X, op=op.add)
                nc.vector.tensor_copy(er1_all[:, g:g + 1], hx1[:, 8:9])
                # hp1 = h1 @ W1 via PE transpose
                pt1 = ppool.tile([128, 128], dt.float32, space="PSUM", tag="pt1")
                nc.tensor.transpose(pt1[:], h1[:, 0:128], id_sb[:])
                pt2 = ppool.tile([128, 128], dt.float32, space="PSUM", tag="pt2")
                nc.tensor.transpose(pt2[0:12, :], h1[:, 128:140], id_sb[:])
                t1s = wpool.tile([128, 128], dt.float32, tag="t1s")
                nc.vector.tensor_copy(t1s[:], pt1[:])
                t2s = wpool.tile([128, 128], dt.float32, tag="t2s")
                nc.vector.tensor_copy(t2s[0:12, :], pt2[0:12, :])
                php = ppool.tile([128, D1], dt.float32, space="PSUM", tag="php")
                nc.tensor.matmul(php[:], t1s[:], w1a_sb[:], start=True, stop=False)
                nc.tensor.matmul(php[:], t2s[0:12, :], w1b_sb[0:12, :],
                                 start=False, stop=True)
                nc.vector.tensor_copy(hx1[:, 0:D1], php[:])
                nc.vector.memset(hx1[:, 9:16], 0.0)
                if g == G - 1:
                    nc.vector.tensor_tensor(hx1[:, 7:8], hx1[:, 7:8],
                                            sm_sb[:], op=op.add)
                nc.sync.dma_start(shard1[:].rearrange("(g p) w -> g p w", p=128)[g],
                                  hx1[:])

            nc.gpsimd.collective_compute(
                "AllGather", op.bypass, replica_groups=[list(range(NCORES))],
                ins=[shard1[:]], outs=[table1[:]])

            # ---- Phase E1 ----
            for g in range(G):
                K = int(Kg[g])
                gt = gpool.tile([128, K * W1C], dt.float32, tag="g1")
                gv = gt[:].rearrange("p (k w) -> p k w", w=W1C)
                for k in range(K):
                    nc.gpsimd.indirect_dma_start(
                        out=gv[:, k], out_offset=None, in_=table1[:],
                        in_offset=bass.IndirectOffsetOnAxis(
                            ap=of1_sb[:, cums[g] + k:cums[g] + k + 1], axis=0))
                ep = spool.tile([128, K], dt.float32, tag="ep1")
                nc.vector.tensor_scalar(ep[:], gv[:, :, 7], er1_all[:, g:g + 1],
                                        None, op.add)
                ee = spool.tile([128, K], dt.float32, tag="ee1")
                nc.vector.scalar_tensor_tensor(
                    out=ee[:], in0=ep[:], scalar=0.2, in1=ep[:],
                    op0=op.mult, op1=op.max)
                ex = spool.tile([128, K], dt.float32, tag="ex1")
                dn = spool.tile([128, 1], dt.float32, tag="dn1")
                nc.scalar.activation(ex[:], ee[:], act.Exp, accum_out=dn[:])
                nc.vector.tensor_scalar_max(dn[:], dn[:], 1e-30)
                rv = spool.tile([128, 1], dt.float32, tag="rv1")
                nc.vector.reciprocal(rv[:], dn[:])
                acc = spool.tile([128, D1], dt.float32, tag="acc1")
                nc.vector.tensor_scalar(acc[:], gv[:, 0, 0:D1], ex[:, 0:1], None,
                                        op.mult)
                for k in range(1, K):
                    nc.vector.scalar_tensor_tensor(
                        out=acc[:], in0=gv[:, k, 0:D1], scalar=ex[:, k:k + 1],
                        in1=acc[:], op0=op.mult, op1=op.add)
                ot = spool.tile([128, D1], dt.float32, tag="ot")
                nc.vector.scalar_tensor_tensor(
                    out=ot[:], in0=acc[:], scalar=rv[:], in1=b1_sb[:],
                    op0=op.mult, op1=op.add)
                nc.scalar.activation(ot[:], ot[:], act.Relu)
                nc.sync.dma_start(t_out[:].rearrange("(g p) w -> g p w", p=128)[g],
                                  ot[:])
    nc.compile()
    return nc


def kernel(**inputs):
    percore, Kg, cums = _host_prep(inputs["src"], inputs["dst"])
    com, xts = _prep_weights(inputs)

    key = tuple(Kg)
    if key not in _CACHE:
        _CACHE[key] = _build(Kg, cums)
    nc = _CACHE[key]

    in_maps = []
    for c in range(NCORES):
        pc = percore[c]
        m = dict(x_up=xts[c], w0r=com["W0r"], w1a=com["W1a"], w1b=com["W1b"],
                 al0b=com["al0b"], ar0b=com["ar0b"], b0b=com["b0b"],
                 wal1b=com["wal1b"], war1b=com["war1b"], b1b=com["b1b"],
                 ident=com["ident"], offs0=pc["offs0"], offs1=pc["offs1"],
                 dst_er=pc["dst_er"], sent_mask=com["sent_mask"])
        in_maps.append(m)

    from concourse.bass_utils import run_bass_kernel_spmd
    global LAST_EXEC_NS
    res = run_bass_kernel_spmd(nc, in_maps, core_ids=list(range(NCORES)),
                               trace=TRACE)
    LAST_EXEC_NS = res.exec_time_ns
    out = np.zeros((N, D1), dtype=np.float32)
    for c in range(NCORES):
        ob = res.results[c]["out_buf"]
        out[c * SHARD + percore[c]["order"]] = ob[:SHARD]
    return out


# revision 27
# speedup vs baseline: 2246.3100x; 1.0484x over previous
"""Trainium2 Bass kernel for 2-layer single-head GAT (nn_GAT__80942953660642).

Strategy (8 NeuronCores, SPMD):
  - Nodes sharded contiguously: core c owns nodes [c*12500, (c+1)*12500).
  - Phase M: h = X_shard @ W0 on PE (host-pretransposed X tiles), el/er via
    DVE reduces; rows [h(140), el, er, pad] packed into a 144-f32 shard table.
  - AllGather the 7.2MB shard tables -> full 57.8MB node table per core.
  - Phase E0 (edge phase): per core, its dst nodes are degree-sorted into 98
    groups of 128 (one dst per partition). Each dst's incoming edges occupy
    padded slot columns; slot gathers use per-partition indirect DMA (128
    rows/call, int32 element offsets). Edge softmax without max-subtraction
    (numerically safe here); padding slots point at sentinel rows with
    el = -1e30 so exp() kills them. Weighted accumulation via fused DVE
    multiply-add over slot columns.
  - hp1 = h1 @ W1 (PE transpose + matmul), second 16-f32 table, AllGather,
    Phase E1 repeats the edge phase at width 7.
  - Host assembles the final [100000, 7] output (inverse degree-sort).
"""
import sys
sys.path.insert(0, "/opt/trn_rl_repo")
import numpy as np

N = 100000
NCORES = 8
SHARD = 12500
PSHARD = 12544          # 98 * 128
G = PSHARD // 128       # 98 groups
KDIM = 1536             # 1433 padded to 12*128
D0 = 140
D1 = 7
W0C = 144               # L0 table row: h(140), el(140), er(141), pad
W1C = 16                # L1 table row: hp1(7), el1(7), er1(8), pad
SENT = np.float32(-1e30)

_CACHE = {}
TRACE = False          # test harness sets this to capture an NTFF profile
LAST_EXEC_NS = None


def _host_prep(src, dst):
    src = np.asarray(src).astype(np.int64)
    dst = np.asarray(dst).astype(np.int64)
    deg = np.bincount(dst, minlength=N)
    nodes = np.arange(N, dtype=np.int64)
    pad_id = (nodes // SHARD) * PSHARD + (nodes % SHARD)  # original-order padded id

    percore = []
    Kg = np.zeros(G, dtype=np.int64)
    for c in range(NCORES):
        lo = c * SHARD
        m = (dst >= lo) & (dst < lo + SHARD)
        e_dst = dst[m] - lo
        e_src = src[m]
        d = deg[lo:lo + SHARD]
        order = np.argsort(-d, kind="stable")
        rank = np.empty(SHARD, dtype=np.int64)
        rank[order] = np.arange(SHARD)
        dsort = np.concatenate([d[order], np.zeros(PSHARD - SHARD, np.int64)])
        for g in range(G):
            Kg[g] = max(Kg[g], max(1, dsort[g * 128:(g + 1) * 128].max()))
        percore.append(dict(order=order, rank=rank, e_dst=e_dst, e_src=e_src))

    pos1 = np.empty(N, dtype=np.int64)  # sorted-space padded id
    for c in range(NCORES):
        pos1[c * SHARD:(c + 1) * SHARD] = c * PSHARD + percore[c]["rank"]

    SK = int(Kg.sum())
    cums = np.concatenate([[0], np.cumsum(Kg)])
    for c in range(NCORES):
        pc = percore[c]
        sent_row = c * PSHARD + PSHARD - 1
        r = pc["rank"][pc["e_dst"]]
        # self-loop edges first within each dst -> they land in slot 0
        not_self = (pc["e_src"] != pc["e_dst"] + c * SHARD).astype(np.int64)
        ordr = np.lexsort((not_self, r))
        r_s = r[ordr]
        kpos = np.arange(len(r_s)) - np.searchsorted(r_s, r_s)
        Kcap = int(Kg.max())
        slots0 = np.full((PSHARD, Kcap), sent_row, dtype=np.int64)
        slots0[r_s, kpos] = pad_id[pc["e_src"][ordr]]
        slots1 = np.full((PSHARD, Kcap), sent_row, dtype=np.int64)
        slots1[r_s, kpos] = pos1[pc["e_src"][ordr]]
        # pack per-group [128, Kg[g]] -> [128, SK] (element offsets)
        offs0 = np.zeros((128, SK), np.int32)
        offs1 = np.zeros((128, SK), np.int32)
        for g in range(G):
            offs0[:, cums[g]:cums[g + 1]] = slots0[g * 128:(g + 1) * 128, :Kg[g]]
            offs1[:, cums[g]:cums[g + 1]] = slots1[g * 128:(g + 1) * 128, :Kg[g]]
        pc["offs0"] = offs0
        pc["offs1"] = offs1
    return percore, Kg.astype(int), cums.astype(int)


def _prep_weights(inputs):
    X = np.asarray(inputs["X"], np.float32)
    W0 = np.asarray(inputs["W0"], np.float32)
    al0 = np.asarray(inputs["al0"], np.float32)
    ar0 = np.asarray(inputs["ar0"], np.float32)
    b0 = np.asarray(inputs["b0"], np.float32)
    W1 = np.asarray(inputs["W1"], np.float32)
    al1 = np.asarray(inputs["al1"], np.float32)
    ar1 = np.asarray(inputs["ar1"], np.float32)
    b1 = np.asarray(inputs["b1"], np.float32)

    # W0 rearranged: [128, 12*140]; W0r[kp, k*140+j] = W0[k*128+kp, j]
    W0p = np.zeros((KDIM, D0), np.float32)
    W0p[:1433] = W0
    W0r = np.ascontiguousarray(
        W0p.reshape(12, 128, D0).transpose(1, 0, 2).reshape(128, 12 * D0))
    W1a = np.zeros((128, D1), np.float32)
    W1a[:128] = W1[:128]
    W1b = np.zeros((128, D1), np.float32)
    W1b[:12] = W1[128:140]
    bc = lambda v, w: np.broadcast_to(np.asarray(v, np.float32)[None, :],
                                      (128, w)).copy()
    wal1 = W1 @ al1
    war1 = W1 @ ar1
    ident = np.eye(128, dtype=np.float32)
    sent_mask = np.zeros((128, 1), np.float32)
    sent_mask[SHARD - (G - 1) * 128:, 0] = SENT   # partitions 84.. are pads
    com = dict(W0r=W0r, W1a=W1a, W1b=W1b, sent_mask=sent_mask,
               al0b=bc(al0, D0), ar0b=bc(ar0, D0), b0b=bc(b0, D0),
               wal1b=bc(wal1, D0), war1b=bc(war1, D0), b1b=bc(b1, D1),
               ident=ident)

    # X tiles per core: xt[n, kp, k*128+nf] = X[lo + n*128+nf, k*128+kp]
    xts = []
    for c in range(NCORES):
        lo = c * SHARD
        Xp = np.zeros((PSHARD, KDIM), np.float32)
        Xp[:SHARD, :1433] = X[lo:lo + SHARD, :]
        xt = np.ascontiguousarray(
            Xp.reshape(G, 128, 12, 128).transpose(0, 3, 2, 1).reshape(G, 128, 12 * 128))
        xts.append(xt)
    return com, xts


def _build(Kg, cums):
    import concourse.bass as bass
    import concourse.tile as tile
    from concourse import bacc, mybir
    dt = mybir.dt
    op = mybir.AluOpType
    act = mybir.ActivationFunctionType

    SK = int(sum(Kg))
    nc = bacc.Bacc("TRN2", target_bir_lowering=False, debug=False,
                   num_devices=NCORES)
    t_x = nc.dram_tensor("x_up", [G, 128, 12 * 128], dt.float32, kind="ExternalInput")
    t_w0 = nc.dram_tensor("w0r", [128, 12 * D0], dt.float32, kind="ExternalInput")
    t_w1a = nc.dram_tensor("w1a", [128, D1], dt.float32, kind="ExternalInput")
    t_w1b = nc.dram_tensor("w1b", [128, D1], dt.float32, kind="ExternalInput")
    t_al0 = nc.dram_tensor("al0b", [128, D0], dt.float32, kind="ExternalInput")
    t_ar0 = nc.dram_tensor("ar0b", [128, D0], dt.float32, kind="ExternalInput")
    t_b0 = nc.dram_tensor("b0b", [128, D0], dt.float32, kind="ExternalInput")
    t_wal1 = nc.dram_tensor("wal1b", [128, D0], dt.float32, kind="ExternalInput")
    t_war1 = nc.dram_tensor("war1b", [128, D0], dt.float32, kind="ExternalInput")
    t_b1 = nc.dram_tensor("b1b", [128, D1], dt.float32, kind="ExternalInput")
    t_id = nc.dram_tensor("ident", [128, 128], dt.float32, kind="ExternalInput")
    t_of0 = nc.dram_tensor("offs0", [128, SK], dt.int32, kind="ExternalInput")
    t_of1 = nc.dram_tensor("offs1", [128, SK], dt.int32, kind="ExternalInput")
    t_sm = nc.dram_tensor("sent_mask", [128, 1], dt.float32, kind="ExternalInput")
    t_out = nc.dram_tensor("out_buf", [PSHARD, D1], dt.float32, kind="ExternalOutput")

    with tile.TileContext(nc) as tc:
        with tc.tile_pool(name="const", bufs=1) as cpool, \
             tc.tile_pool(name="xload", bufs=2) as xpool, \
             tc.tile_pool(name="hex", bufs=3) as hexpool, \
             tc.tile_pool(name="gath", bufs=2) as gpool, \
             tc.tile_pool(name="work", bufs=3) as wpool, \
             tc.tile_pool(name="small", bufs=4) as spool, \
             tc.tile_pool(name="psum", bufs=2, space="PSUM") as ppool, \
             tc.tile_pool(name="dram", bufs=1, space="DRAM") as dpool:

            # constants
            w0_sb = cpool.tile([128, 12 * D0], dt.float32)
            nc.sync.dma_start(w0_sb[:], t_w0[:])
            w1a_sb = cpool.tile([128, D1], dt.float32)
            nc.sync.dma_start(w1a_sb[:], t_w1a[:])
            w1b_sb = cpool.tile([128, D1], dt.float32)
            nc.sync.dma_start(w1b_sb[:], t_w1b[:])
            al0_sb = cpool.tile([128, D0], dt.float32)
            nc.sync.dma_start(al0_sb[:], t_al0[:])
            ar0_sb = cpool.tile([128, D0], dt.float32)
            nc.sync.dma_start(ar0_sb[:], t_ar0[:])
            b0_sb = cpool.tile([128, D0], dt.float32)
            nc.sync.dma_start(b0_sb[:], t_b0[:])
            wal1_sb = cpool.tile([128, D0], dt.float32)
            nc.sync.dma_start(wal1_sb[:], t_wal1[:])
            war1_sb = cpool.tile([128, D0], dt.float32)
            nc.sync.dma_start(war1_sb[:], t_war1[:])
            b1_sb = cpool.tile([128, D1], dt.float32)
            nc.sync.dma_start(b1_sb[:], t_b1[:])
            id_sb = cpool.tile([128, 128], dt.float32)
            nc.sync.dma_start(id_sb[:], t_id[:])
            of0_sb = cpool.tile([128, SK], dt.int32)
            nc.sync.dma_start(of0_sb[:], t_of0[:])
            of1_sb = cpool.tile([128, SK], dt.int32)
            nc.sync.dma_start(of1_sb[:], t_of1[:])
            sm_sb = cpool.tile([128, 1], dt.float32)
            nc.sync.dma_start(sm_sb[:], t_sm[:])
            er1_all = cpool.tile([128, G], dt.float32)

            shard0 = dpool.tile([PSHARD, W0C], dt.float32)
            table0 = dpool.tile([NCORES * PSHARD, W0C], dt.float32, addr_space="Shared")
            shard1 = dpool.tile([PSHARD, W1C], dt.float32)
            table1 = dpool.tile([NCORES * PSHARD, W1C], dt.float32, addr_space="Shared")

            # ---- Phase M: h = X @ W0, pack [h, el, er] rows ----
            for n in range(G):
                xt = xpool.tile([128, 12 * 128], dt.float32)
                nc.sync.dma_start(xt[:], t_x[:][n])
                ph = ppool.tile([128, D0], dt.float32, space="PSUM")
                for k in range(12):
                    nc.tensor.matmul(ph[:], xt[:, k * 128:(k + 1) * 128],
                                     w0_sb[:, k * D0:(k + 1) * D0],
                                     start=(k == 0), stop=(k == 11))
                hx = hexpool.tile([128, W0C], dt.float32, tag="hex0")
                nc.vector.tensor_copy(hx[:, 0:D0], ph[:])
                scr = wpool.tile([128, D0], dt.float32, tag="mscr")
                nc.vector.tensor_tensor(scr[:], ph[:], al0_sb[:], op=op.mult)
                nc.vector.tensor_reduce(hx[:, 140:141], scr[:],
                                        axis=mybir.AxisListType.X, op=op.add)
                nc.vector.tensor_tensor(scr[:], ph[:], ar0_sb[:], op=op.mult)
                nc.vector.tensor_reduce(hx[:, 141:142], scr[:],
                                        axis=mybir.AxisListType.X, op=op.add)
                nc.vector.memset(hx[:, 142:144], 0.0)
                if n == G - 1:
                    nc.vector.tensor_tensor(hx[:, 140:141], hx[:, 140:141],
                                            sm_sb[:], op=op.add)
                nc.sync.dma_start(shard0[:].rearrange("(g p) w -> g p w", p=128)[n],
                                  hx[:])

            nc.gpsimd.collective_compute(
                "AllGather", op.bypass, replica_groups=[list(range(NCORES))],
                ins=[shard0[:]], outs=[table0[:]])

            # ---- Phase E0 ----
            for g in range(G):
                K = int(Kg[g])
                gt = gpool.tile([128, K * W0C], dt.float32, tag="g0")
                gv = gt[:].rearrange("p (k w) -> p k w", w=W0C)
                for k in range(K):
                    nc.gpsimd.indirect_dma_start(
                        out=gv[:, k], out_offset=None, in_=table0[:],
                        in_offset=bass.IndirectOffsetOnAxis(
                            ap=of0_sb[:, cums[g] + k:cums[g] + k + 1], axis=0))
                # slot 0 is the self-loop -> its row IS the dst row; er = col 141
                ep = spool.tile([128, K], dt.float32, tag="ep0")
                nc.vector.tensor_scalar(ep[:], gv[:, :, 140], gv[:, 0, 141:142],
                                        None, op.add)
                ee = spool.tile([128, K], dt.float32, tag="ee0")
                nc.vector.scalar_tensor_tensor(
                    out=ee[:], in0=ep[:], scalar=0.2, in1=ep[:],
                    op0=op.mult, op1=op.max)
                ex = spool.tile([128, K], dt.float32, tag="ex0")
                dn = spool.tile([128, 1], dt.float32, tag="dn0")
                nc.scalar.activation(ex[:], ee[:], act.Exp, accum_out=dn[:])
                nc.vector.tensor_scalar_max(dn[:], dn[:], 1e-30)
                rv = spool.tile([128, 1], dt.float32, tag="rv0")
                nc.vector.reciprocal(rv[:], dn[:])
                acc = wpool.tile([128, D0], dt.float32, tag="acc0")
                nc.vector.tensor_scalar(acc[:], gv[:, 0, 0:D0], ex[:, 0:1], None,
                                        op.mult)
                for k in range(1, K):
                    nc.vector.scalar_tensor_tensor(
                        out=acc[:], in0=gv[:, k, 0:D0], scalar=ex[:, k:k + 1],
                        in1=acc[:], op0=op.mult, op1=op.add)
                h1 = wpool.tile([128, D0], dt.float32, tag="h1")
                nc.vector.scalar_tensor_tensor(
                    out=h1[:], in0=acc[:], scalar=rv[:], in1=b0_sb[:],
                    op0=op.mult, op1=op.add)
                nc.scalar.activation(h1[:], h1[:], act.Relu)
                # el1/er1
                hx1 = hexpool.tile([128, W1C], dt.float32, tag="hex1")
                scr1 = wpool.tile([128, D0], dt.float32, tag="escr")
                nc.vector.tensor_tensor(scr1[:], h1[:], wal1_sb[:], op=op.mult)
                nc.vector.tensor_reduce(hx1[:, 7:8], scr1[:],
                                        axis=mybir.AxisListType.X, op=op.add)
                nc.vector.tensor_tensor(scr1[:], h1[:], war1_sb[:], op=op.mult)
                nc.vector.tensor_reduce(hx1[:, 8:9], scr1[:],
                                        axis=mybir.AxisListType.X, op=op.add)
                nc.vector.tensor_copy(er1_all[:, g:g + 1], hx1[:, 8:9])
                # hp1 = h1 @ W1 via PE transpose
                pt1 = ppool.tile([128, 128], dt.float32, space="PSUM", tag="pt1")
                nc.tensor.transpose(pt1[:], h1[:, 0:128], id_sb[:])
                pt2 = ppool.tile([128, 128], dt.float32, space="PSUM", tag="pt2")
                nc.tensor.transpose(pt2[0:12, :], h1[:, 128:140], id_sb[:])
                t1s = wpool.tile([128, 128], dt.float32, tag="t1s")
                nc.vector.tensor_copy(t1s[:], pt1[:])
                t2s = wpool.tile([128, 128], dt.float32, tag="t2s")
                nc.vector.tensor_copy(t2s[0:12, :], pt2[0:12, :])
                php = ppool.tile([128, D1], dt.float32, space="PSUM", tag="php")
                nc.tensor.matmul(php[:], t1s[:], w1a_sb[:], start=True, stop=False)
                nc.tensor.matmul(php[:], t2s[0:12, :], w1b_sb[0:12, :],
                                 start=False, stop=True)
                nc.vector.tensor_copy(hx1[:, 0:D1], php[:])
                nc.vector.memset(hx1[:, 9:16], 0.0)
                if g == G - 1:
                    nc.vector.tensor_tensor(hx1[:, 7:8], hx1[:, 7:8],
                                            sm_sb[:], op=op.add)
                nc.sync.dma_start(shard1[:].rearrange("(g p) w -> g p w", p=128)[g],
                                  hx1[:])

            nc.gpsimd.collective_compute(
                "AllGather", op.bypass, replica_groups=[list(range(NCORES))],
                ins=[shard1[:]], outs=[table1[:]])

            # ---- Phase E1 ----
            for g in range(G):
                K = int(Kg[g])
                gt = gpool.tile([128, K * W1C], dt.float32, tag="g1")
                gv = gt[:].rearrange("p (k w) -> p k w", w=W1C)
                # slot 0 = self-loop: contiguous rows of our own shard (sorted space)
                nc.sync.dma_start(
                    gv[:, 0], shard1[:].rearrange("(g p) w -> g p w", p=128)[g])
                for k in range(1, K):
                    nc.gpsimd.indirect_dma_start(
                        out=gv[:, k], out_offset=None, in_=table1[:],
                        in_offset=bass.IndirectOffsetOnAxis(
                            ap=of1_sb[:, cums[g] + k:cums[g] + k + 1], axis=0))
                ep = spool.tile([128, K], dt.float32, tag="ep1")
                nc.vector.tensor_scalar(ep[:], gv[:, :, 7], er1_all[:, g:g + 1],
                                        None, op.add)
                ee = spool.tile([128, K], dt.float32, tag="ee1")
                nc.vector.scalar_tensor_tensor(
                    out=ee[:], in0=ep[:], scalar=0.2, in1=ep[:],
                    op0=op.mult, op1=op.max)
                ex = spool.tile([128, K], dt.float32, tag="ex1")
                dn = spool.tile([128, 1], dt.float32, tag="dn1")
                nc.scalar.activation(ex[:], ee[:], act.Exp, accum_out=dn[:])
                nc.vector.tensor_scalar_max(dn[:], dn[:], 1e-30)
                rv = spool.tile([128, 1], dt.float32, tag="rv1")
                nc.vector.reciprocal(rv[:], dn[:])
                acc = spool.tile([128, D1], dt.float32, tag="acc1")
                nc.vector.tensor_scalar(acc[:], gv[:, 0, 0:D1], ex[:, 0:1], None,
                                        op.mult)
                for k in range(1, K):
                    nc.vector.scalar_tensor_tensor(
                        out=acc[:], in0=gv[:, k, 0:D1], scalar=ex[:, k:k + 1],
                        in1=acc[:], op0=op.mult, op1=op.add)
                ot = spool.tile([128, D1], dt.float32, tag="ot")
                nc.vector.scalar_tensor_tensor(
                    out=ot[:], in0=acc[:], scalar=rv[:], in1=b1_sb[:],
                    op0=op.mult, op1=op.add)
                nc.scalar.activation(ot[:], ot[:], act.Relu)
                nc.sync.dma_start(t_out[:].rearrange("(g p) w -> g p w", p=128)[g],
                                  ot[:])
    nc.compile()
    return nc


def kernel(**inputs):
    percore, Kg, cums = _host_prep(inputs["src"], inputs["dst"])
    com, xts = _prep_weights(inputs)

    key = tuple(Kg)
    if key not in _CACHE:
        _CACHE[key] = _build(Kg, cums)
    nc = _CACHE[key]

    in_maps = []
    for c in range(NCORES):
        pc = percore[c]
        m = dict(x_up=xts[c], w0r=com["W0r"], w1a=com["W1a"], w1b=com["W1b"],
                 al0b=com["al0b"], ar0b=com["ar0b"], b0b=com["b0b"],
                 wal1b=com["wal1b"], war1b=com["war1b"], b1b=com["b1b"],
                 ident=com["ident"], offs0=pc["offs0"], offs1=pc["offs1"],
                 sent_mask=com["sent_mask"])
        in_maps.append(m)

    from concourse.bass_utils import run_bass_kernel_spmd
    global LAST_EXEC_NS
    res = run_bass_kernel_spmd(nc, in_maps, core_ids=list(range(NCORES)),
                               trace=TRACE)
    LAST_EXEC_NS = res.exec_time_ns
    out = np.zeros((N, D1), dtype=np.float32)
    for c in range(NCORES):
        ob = res.results[c]["out_buf"]
        out[c * SHARD + percore[c]["order"]] = ob[:SHARD]
    return out


# revision 28
# speedup vs baseline: 2246.5022x; 1.0001x over previous
"""Trainium2 Bass kernel for 2-layer single-head GAT (nn_GAT__80942953660642).

Strategy (8 NeuronCores, SPMD):
  - Nodes sharded contiguously: core c owns nodes [c*12500, (c+1)*12500).
  - Phase M: h = X_shard @ W0 on PE (host-pretransposed X tiles), el/er via
    DVE reduces; rows [h(140), el, er, pad] packed into a 144-f32 shard table.
  - AllGather the 7.2MB shard tables -> full 57.8MB node table per core.
  - Phase E0 (edge phase): per core, its dst nodes are degree-sorted into 98
    groups of 128 (one dst per partition). Each dst's incoming edges occupy
    padded slot columns; slot gathers use per-partition indirect DMA (128
    rows/call, int32 element offsets). Edge softmax without max-subtraction
    (numerically safe here); padding slots point at sentinel rows with
    el = -1e30 so exp() kills them. Weighted accumulation via fused DVE
    multiply-add over slot columns.
  - hp1 = h1 @ W1 (PE transpose + matmul), second 16-f32 table, AllGather,
    Phase E1 repeats the edge phase at width 7.
  - Host assembles the final [100000, 7] output (inverse degree-sort).
"""
import sys
sys.path.insert(0, "/opt/trn_rl_repo")
import numpy as np

N = 100000
NCORES = 8
SHARD = 12500
PSHARD = 12544          # 98 * 128
G = PSHARD // 128       # 98 groups
KDIM = 1536             # 1433 padded to 12*128
D0 = 140
D1 = 7
W0C = 144               # L0 table row: h(140), el(140), er(141), pad
W1C = 16                # L1 table row: hp1(7), el1(7), er1(8), pad
SENT = np.float32(-1e30)

_CACHE = {}
TRACE = False          # test harness sets this to capture an NTFF profile
LAST_EXEC_NS = None


def _host_prep(src, dst):
    src = np.asarray(src).astype(np.int64)
    dst = np.asarray(dst).astype(np.int64)
    deg = np.bincount(dst, minlength=N)
    nodes = np.arange(N, dtype=np.int64)
    pad_id = (nodes // SHARD) * PSHARD + (nodes % SHARD)  # original-order padded id

    percore = []
    Kg = np.zeros(G, dtype=np.int64)
    for c in range(NCORES):
        lo = c * SHARD
        m = (dst >= lo) & (dst < lo + SHARD)
        e_dst = dst[m] - lo
        e_src = src[m]
        d = deg[lo:lo + SHARD]
        order = np.argsort(-d, kind="stable")
        rank = np.empty(SHARD, dtype=np.int64)
        rank[order] = np.arange(SHARD)
        dsort = np.concatenate([d[order], np.zeros(PSHARD - SHARD, np.int64)])
        for g in range(G):
            Kg[g] = max(Kg[g], max(1, dsort[g * 128:(g + 1) * 128].max()))
        percore.append(dict(order=order, rank=rank, e_dst=e_dst, e_src=e_src))

    pos1 = np.empty(N, dtype=np.int64)  # sorted-space padded id
    for c in range(NCORES):
        pos1[c * SHARD:(c + 1) * SHARD] = c * PSHARD + percore[c]["rank"]

    SK = int(Kg.sum())
    cums = np.concatenate([[0], np.cumsum(Kg)])
    for c in range(NCORES):
        pc = percore[c]
        sent_row = c * PSHARD + PSHARD - 1
        r = pc["rank"][pc["e_dst"]]
        # self-loop edges first within each dst -> they land in slot 0
        not_self = (pc["e_src"] != pc["e_dst"] + c * SHARD).astype(np.int64)
        ordr = np.lexsort((not_self, r))
        r_s = r[ordr]
        kpos = np.arange(len(r_s)) - np.searchsorted(r_s, r_s)
        Kcap = int(Kg.max())
        slots0 = np.full((PSHARD, Kcap), sent_row, dtype=np.int64)
        slots0[r_s, kpos] = pad_id[pc["e_src"][ordr]]
        slots1 = np.full((PSHARD, Kcap), sent_row, dtype=np.int64)
        slots1[r_s, kpos] = pos1[pc["e_src"][ordr]]
        # pack per-group [128, Kg[g]] -> [128, SK] (element offsets)
        offs0 = np.zeros((128, SK), np.int32)
        offs1 = np.zeros((128, SK), np.int32)
        for g in range(G):
            offs0[:, cums[g]:cums[g + 1]] = slots0[g * 128:(g + 1) * 128, :Kg[g]]
            offs1[:, cums[g]:cums[g + 1]] = slots1[g * 128:(g + 1) * 128, :Kg[g]]
        pc["offs0"] = offs0
        pc["offs1"] = offs1
    return percore, Kg.astype(int), cums.astype(int)


def _prep_weights(inputs):
    X = np.asarray(inputs["X"], np.float32)
    W0 = np.asarray(inputs["W0"], np.float32)
    al0 = np.asarray(inputs["al0"], np.float32)
    ar0 = np.asarray(inputs["ar0"], np.float32)
    b0 = np.asarray(inputs["b0"], np.float32)
    W1 = np.asarray(inputs["W1"], np.float32)
    al1 = np.asarray(inputs["al1"], np.float32)
    ar1 = np.asarray(inputs["ar1"], np.float32)
    b1 = np.asarray(inputs["b1"], np.float32)

    # W0 rearranged: [128, 12*140]; W0r[kp, k*140+j] = W0[k*128+kp, j]
    W0p = np.zeros((KDIM, D0), np.float32)
    W0p[:1433] = W0
    W0r = np.ascontiguousarray(
        W0p.reshape(12, 128, D0).transpose(1, 0, 2).reshape(128, 12 * D0))
    W1a = np.zeros((128, D1), np.float32)
    W1a[:128] = W1[:128]
    W1b = np.zeros((128, D1), np.float32)
    W1b[:12] = W1[128:140]
    bc = lambda v, w: np.broadcast_to(np.asarray(v, np.float32)[None, :],
                                      (128, w)).copy()
    wal1 = W1 @ al1
    war1 = W1 @ ar1
    ident = np.eye(128, dtype=np.float32)
    sent_mask = np.zeros((128, 1), np.float32)
    sent_mask[SHARD - (G - 1) * 128:, 0] = SENT   # partitions 84.. are pads
    com = dict(W0r=W0r, W1a=W1a, W1b=W1b, sent_mask=sent_mask,
               al0b=bc(al0, D0), ar0b=bc(ar0, D0), b0b=bc(b0, D0),
               wal1b=bc(wal1, D0), war1b=bc(war1, D0), b1b=bc(b1, D1),
               ident=ident)

    # X tiles per core: xt[n, kp, k*128+nf] = X[lo + n*128+nf, k*128+kp]
    xts = []
    for c in range(NCORES):
        lo = c * SHARD
        Xp = np.zeros((PSHARD, KDIM), np.float32)
        Xp[:SHARD, :1433] = X[lo:lo + SHARD, :]
        xt = np.ascontiguousarray(
            Xp.reshape(G, 128, 12, 128).transpose(0, 3, 2, 1).reshape(G, 128, 12 * 128))
        xts.append(xt)
    return com, xts


def _build(Kg, cums):
    import concourse.bass as bass
    import concourse.tile as tile
    from concourse import bacc, mybir
    dt = mybir.dt
    op = mybir.AluOpType
    act = mybir.ActivationFunctionType

    SK = int(sum(Kg))
    nc = bacc.Bacc("TRN2", target_bir_lowering=False, debug=False,
                   num_devices=NCORES)
    t_x = nc.dram_tensor("x_up", [G, 128, 12 * 128], dt.float32, kind="ExternalInput")
    t_w0 = nc.dram_tensor("w0r", [128, 12 * D0], dt.float32, kind="ExternalInput")
    t_w1a = nc.dram_tensor("w1a", [128, D1], dt.float32, kind="ExternalInput")
    t_w1b = nc.dram_tensor("w1b", [128, D1], dt.float32, kind="ExternalInput")
    t_al0 = nc.dram_tensor("al0b", [128, D0], dt.float32, kind="ExternalInput")
    t_ar0 = nc.dram_tensor("ar0b", [128, D0], dt.float32, kind="ExternalInput")
    t_b0 = nc.dram_tensor("b0b", [128, D0], dt.float32, kind="ExternalInput")
    t_wal1 = nc.dram_tensor("wal1b", [128, D0], dt.float32, kind="ExternalInput")
    t_war1 = nc.dram_tensor("war1b", [128, D0], dt.float32, kind="ExternalInput")
    t_b1 = nc.dram_tensor("b1b", [128, D1], dt.float32, kind="ExternalInput")
    t_id = nc.dram_tensor("ident", [128, 128], dt.float32, kind="ExternalInput")
    t_of0 = nc.dram_tensor("offs0", [128, SK], dt.int32, kind="ExternalInput")
    t_of1 = nc.dram_tensor("offs1", [128, SK], dt.int32, kind="ExternalInput")
    t_sm = nc.dram_tensor("sent_mask", [128, 1], dt.float32, kind="ExternalInput")
    t_out = nc.dram_tensor("out_buf", [PSHARD, D1], dt.float32, kind="ExternalOutput")

    with tile.TileContext(nc) as tc:
        with tc.tile_pool(name="const", bufs=1) as cpool, \
             tc.tile_pool(name="xload", bufs=2) as xpool, \
             tc.tile_pool(name="hex", bufs=3) as hexpool, \
             tc.tile_pool(name="gath", bufs=4) as gpool, \
             tc.tile_pool(name="work", bufs=3) as wpool, \
             tc.tile_pool(name="small", bufs=4) as spool, \
             tc.tile_pool(name="psum", bufs=2, space="PSUM") as ppool, \
             tc.tile_pool(name="dram", bufs=1, space="DRAM") as dpool:

            # constants
            w0_sb = cpool.tile([128, 12 * D0], dt.float32)
            nc.sync.dma_start(w0_sb[:], t_w0[:])
            w1a_sb = cpool.tile([128, D1], dt.float32)
            nc.sync.dma_start(w1a_sb[:], t_w1a[:])
            w1b_sb = cpool.tile([128, D1], dt.float32)
            nc.sync.dma_start(w1b_sb[:], t_w1b[:])
            al0_sb = cpool.tile([128, D0], dt.float32)
            nc.sync.dma_start(al0_sb[:], t_al0[:])
            ar0_sb = cpool.tile([128, D0], dt.float32)
            nc.sync.dma_start(ar0_sb[:], t_ar0[:])
            b0_sb = cpool.tile([128, D0], dt.float32)
            nc.sync.dma_start(b0_sb[:], t_b0[:])
            wal1_sb = cpool.tile([128, D0], dt.float32)
            nc.sync.dma_start(wal1_sb[:], t_wal1[:])
            war1_sb = cpool.tile([128, D0], dt.float32)
            nc.sync.dma_start(war1_sb[:], t_war1[:])
            b1_sb = cpool.tile([128, D1], dt.float32)
            nc.sync.dma_start(b1_sb[:], t_b1[:])
            id_sb = cpool.tile([128, 128], dt.float32)
            nc.sync.dma_start(id_sb[:], t_id[:])
            of0_sb = cpool.tile([128, SK], dt.int32)
            nc.sync.dma_start(of0_sb[:], t_of0[:])
            of1_sb = cpool.tile([128, SK], dt.int32)
            nc.sync.dma_start(of1_sb[:], t_of1[:])
            sm_sb = cpool.tile([128, 1], dt.float32)
            nc.sync.dma_start(sm_sb[:], t_sm[:])
            er1_all = cpool.tile([128, G], dt.float32)

            shard0 = dpool.tile([PSHARD, W0C], dt.float32)
            table0 = dpool.tile([NCORES * PSHARD, W0C], dt.float32, addr_space="Shared")
            shard1 = dpool.tile([PSHARD, W1C], dt.float32)
            table1 = dpool.tile([NCORES * PSHARD, W1C], dt.float32, addr_space="Shared")

            # ---- Phase M: h = X @ W0, pack [h, el, er] rows ----
            for n in range(G):
                xt = xpool.tile([128, 12 * 128], dt.float32)
                nc.sync.dma_start(xt[:], t_x[:][n])
                ph = ppool.tile([128, D0], dt.float32, space="PSUM")
                for k in range(12):
                    nc.tensor.matmul(ph[:], xt[:, k * 128:(k + 1) * 128],
                                     w0_sb[:, k * D0:(k + 1) * D0],
                                     start=(k == 0), stop=(k == 11))
                hx = hexpool.tile([128, W0C], dt.float32, tag="hex0")
                nc.vector.tensor_copy(hx[:, 0:D0], ph[:])
                scr = wpool.tile([128, D0], dt.float32, tag="mscr")
                nc.vector.tensor_tensor(scr[:], ph[:], al0_sb[:], op=op.mult)
                nc.vector.tensor_reduce(hx[:, 140:141], scr[:],
                                        axis=mybir.AxisListType.X, op=op.add)
                nc.vector.tensor_tensor(scr[:], ph[:], ar0_sb[:], op=op.mult)
                nc.vector.tensor_reduce(hx[:, 141:142], scr[:],
                                        axis=mybir.AxisListType.X, op=op.add)
                nc.vector.memset(hx[:, 142:144], 0.0)
                if n == G - 1:
                    nc.vector.tensor_tensor(hx[:, 140:141], hx[:, 140:141],
                                            sm_sb[:], op=op.add)
                nc.sync.dma_start(shard0[:].rearrange("(g p) w -> g p w", p=128)[n],
                                  hx[:])

            nc.gpsimd.collective_compute(
                "AllGather", op.bypass, replica_groups=[list(range(NCORES))],
                ins=[shard0[:]], outs=[table0[:]])

            # ---- Phase E0 ----
            for g in range(G):
                K = int(Kg[g])
                gt = gpool.tile([128, K * W0C], dt.float32, tag="g0")
                gv = gt[:].rearrange("p (k w) -> p k w", w=W0C)
                for k in range(K):
                    nc.gpsimd.indirect_dma_start(
                        out=gv[:, k], out_offset=None, in_=table0[:],
                        in_offset=bass.IndirectOffsetOnAxis(
                            ap=of0_sb[:, cums[g] + k:cums[g] + k + 1], axis=0))
                # slot 0 is the self-loop -> its row IS the dst row; er = col 141
                ep = spool.tile([128, K], dt.float32, tag="ep0")
                nc.vector.tensor_scalar(ep[:], gv[:, :, 140], gv[:, 0, 141:142],
                                        None, op.add)
                ee = spool.tile([128, K], dt.float32, tag="ee0")
                nc.vector.scalar_tensor_tensor(
                    out=ee[:], in0=ep[:], scalar=0.2, in1=ep[:],
                    op0=op.mult, op1=op.max)
                ex = spool.tile([128, K], dt.float32, tag="ex0")
                dn = spool.tile([128, 1], dt.float32, tag="dn0")
                nc.scalar.activation(ex[:], ee[:], act.Exp, accum_out=dn[:])
                nc.vector.tensor_scalar_max(dn[:], dn[:], 1e-30)
                rv = spool.tile([128, 1], dt.float32, tag="rv0")
                nc.vector.reciprocal(rv[:], dn[:])
                acc = wpool.tile([128, D0], dt.float32, tag="acc0")
                nc.vector.tensor_scalar(acc[:], gv[:, 0, 0:D0], ex[:, 0:1], None,
                                        op.mult)
                for k in range(1, K):
                    nc.vector.scalar_tensor_tensor(
                        out=acc[:], in0=gv[:, k, 0:D0], scalar=ex[:, k:k + 1],
                        in1=acc[:], op0=op.mult, op1=op.add)
                h1 = wpool.tile([128, D0], dt.float32, tag="h1")
                nc.vector.scalar_tensor_tensor(
                    out=h1[:], in0=acc[:], scalar=rv[:], in1=b0_sb[:],
                    op0=op.mult, op1=op.add)
                nc.scalar.activation(h1[:], h1[:], act.Relu)
                # el1/er1
                hx1 = hexpool.tile([128, W1C], dt.float32, tag="hex1")
                scr1 = wpool.tile([128, D0], dt.float32, tag="escr")
                nc.vector.tensor_tensor(scr1[:], h1[:], wal1_sb[:], op=op.mult)
                nc.vector.tensor_reduce(hx1[:, 7:8], scr1[:],
                                        axis=mybir.AxisListType.X, op=op.add)
                nc.vector.tensor_tensor(scr1[:], h1[:], war1_sb[:], op=op.mult)
                nc.vector.tensor_reduce(hx1[:, 8:9], scr1[:],
                                        axis=mybir.AxisListType.X, op=op.add)
                nc.vector.tensor_copy(er1_all[:, g:g + 1], hx1[:, 8:9])
                # hp1 = h1 @ W1 via PE transpose
                pt1 = ppool.tile([128, 128], dt.float32, space="PSUM", tag="pt1")
                nc.tensor.transpose(pt1[:], h1[:, 0:128], id_sb[:])
                pt2 = ppool.tile([128, 128], dt.float32, space="PSUM", tag="pt2")
                nc.tensor.transpose(pt2[0:12, :], h1[:, 128:140], id_sb[:])
                t1s = wpool.tile([128, 128], dt.float32, tag="t1s")
                nc.vector.tensor_copy(t1s[:], pt1[:])
                t2s = wpool.tile([128, 128], dt.float32, tag="t2s")
                nc.vector.tensor_copy(t2s[0:12, :], pt2[0:12, :])
                php = ppool.tile([128, D1], dt.float32, space="PSUM", tag="php")
                nc.tensor.matmul(php[:], t1s[:], w1a_sb[:], start=True, stop=False)
                nc.tensor.matmul(php[:], t2s[0:12, :], w1b_sb[0:12, :],
                                 start=False, stop=True)
                nc.vector.tensor_copy(hx1[:, 0:D1], php[:])
                nc.vector.memset(hx1[:, 9:16], 0.0)
                if g == G - 1:
                    nc.vector.tensor_tensor(hx1[:, 7:8], hx1[:, 7:8],
                                            sm_sb[:], op=op.add)
                nc.sync.dma_start(shard1[:].rearrange("(g p) w -> g p w", p=128)[g],
                                  hx1[:])

            nc.gpsimd.collective_compute(
                "AllGather", op.bypass, replica_groups=[list(range(NCORES))],
                ins=[shard1[:]], outs=[table1[:]])

            # ---- Phase E1 ----
            for g in range(G):
                K = int(Kg[g])
                gt = gpool.tile([128, K * W1C], dt.float32, tag="g1")
                gv = gt[:].rearrange("p (k w) -> p k w", w=W1C)
                # slot 0 = self-loop: contiguous rows of our own shard (sorted space)
                nc.sync.dma_start(
                    gv[:, 0], shard1[:].rearrange("(g p) w -> g p w", p=128)[g])
                for k in range(1, K):
                    nc.gpsimd.indirect_dma_start(
                        out=gv[:, k], out_offset=None, in_=table1[:],
                        in_offset=bass.IndirectOffsetOnAxis(
                            ap=of1_sb[:, cums[g] + k:cums[g] + k + 1], axis=0))
                ep = spool.tile([128, K], dt.float32, tag="ep1")
                nc.vector.tensor_scalar(ep[:], gv[:, :, 7], er1_all[:, g:g + 1],
                                        None, op.add)
                ee = spool.tile([128, K], dt.float32, tag="ee1")
                nc.vector.scalar_tensor_tensor(
                    out=ee[:], in0=ep[:], scalar=0.2, in1=ep[:],
                    op0=op.mult, op1=op.max)
                ex = spool.tile([128, K], dt.float32, tag="ex1")
                dn = spool.tile([128, 1], dt.float32, tag="dn1")
                nc.scalar.activation(ex[:], ee[:], act.Exp, accum_out=dn[:])
                nc.vector.tensor_scalar_max(dn[:], dn[:], 1e-30)
                rv = spool.tile([128, 1], dt.float32, tag="rv1")
                nc.vector.reciprocal(rv[:], dn[:])
                acc = spool.tile([128, D1], dt.float32, tag="acc1")
                nc.vector.tensor_scalar(acc[:], gv[:, 0, 0:D1], ex[:, 0:1], None,
                                        op.mult)
                for k in range(1, K):
                    nc.vector.scalar_tensor_tensor(
                        out=acc[:], in0=gv[:, k, 0:D1], scalar=ex[:, k:k + 1],
                        in1=acc[:], op0=op.mult, op1=op.add)
                ot = spool.tile([128, D1], dt.float32, tag="ot")
                nc.vector.scalar_tensor_tensor(
                    out=ot[:], in0=acc[:], scalar=rv[:], in1=b1_sb[:],
                    op0=op.mult, op1=op.add)
                nc.scalar.activation(ot[:], ot[:], act.Relu)
                nc.sync.dma_start(t_out[:].rearrange("(g p) w -> g p w", p=128)[g],
                                  ot[:])
    nc.compile()
    return nc


def kernel(**inputs):
    percore, Kg, cums = _host_prep(inputs["src"], inputs["dst"])
    com, xts = _prep_weights(inputs)

    key = tuple(Kg)
    if key not in _CACHE:
        _CACHE[key] = _build(Kg, cums)
    nc = _CACHE[key]

    in_maps = []
    for c in range(NCORES):
        pc = percore[c]
        m = dict(x_up=xts[c], w0r=com["W0r"], w1a=com["W1a"], w1b=com["W1b"],
                 al0b=com["al0b"], ar0b=com["ar0b"], b0b=com["b0b"],
                 wal1b=com["wal1b"], war1b=com["war1b"], b1b=com["b1b"],
                 ident=com["ident"], offs0=pc["offs0"], offs1=pc["offs1"],
                 sent_mask=com["sent_mask"])
        in_maps.append(m)

    from concourse.bass_utils import run_bass_kernel_spmd
    global LAST_EXEC_NS
    res = run_bass_kernel_spmd(nc, in_maps, core_ids=list(range(NCORES)),
                               trace=TRACE)
    LAST_EXEC_NS = res.exec_time_ns
    out = np.zeros((N, D1), dtype=np.float32)
    for c in range(NCORES):
        ob = res.results[c]["out_buf"]
        out[c * SHARD + percore[c]["order"]] = ob[:SHARD]
    return out


# revision 33
# speedup vs baseline: 2280.4936x; 1.0151x over previous
"""Trainium2 Bass kernel for 2-layer single-head GAT (nn_GAT__80942953660642).

Strategy (8 NeuronCores, SPMD):
  - Nodes sharded contiguously: core c owns nodes [c*12500, (c+1)*12500).
  - Phase M: h = X_shard @ W0 on PE (host-pretransposed X tiles), el/er via
    DVE reduces; rows [h(140), el, er, pad] packed into a 144-f32 shard table.
  - AllGather the 7.2MB shard tables -> full 57.8MB node table per core.
  - Phase E0 (edge phase): per core, its dst nodes are degree-sorted into 98
    groups of 128 (one dst per partition). Each dst's incoming edges occupy
    padded slot columns; slot gathers use per-partition indirect DMA (128
    rows/call, int32 element offsets). Edge softmax without max-subtraction
    (numerically safe here); padding slots point at sentinel rows with
    el = -1e30 so exp() kills them. Weighted accumulation via fused DVE
    multiply-add over slot columns.
  - hp1 = h1 @ W1 (PE transpose + matmul), second 16-f32 table, AllGather,
    Phase E1 repeats the edge phase at width 7.
  - Host assembles the final [100000, 7] output (inverse degree-sort).
"""
import sys
sys.path.insert(0, "/opt/trn_rl_repo")
import numpy as np

N = 100000
NCORES = 8
SHARD = 12500
PSHARD = 12544          # 98 * 128
G = PSHARD // 128       # 98 groups
KDIM = 1536             # 1433 padded to 12*128
D0 = 140
D1 = 7
W0C = 144               # L0 table row: h(140), el(140), er(141), pad
W1C = 16                # L1 table row: hp1(7), el1(7), er1(8), pad
SENT = np.float32(-1e30)

_CACHE = {}
TRACE = False          # test harness sets this to capture an NTFF profile
LAST_EXEC_NS = None


def _host_prep(src, dst):
    src = np.asarray(src).astype(np.int64)
    dst = np.asarray(dst).astype(np.int64)
    deg = np.bincount(dst, minlength=N)
    nodes = np.arange(N, dtype=np.int64)
    pad_id = (nodes // SHARD) * PSHARD + (nodes % SHARD)  # original-order padded id

    percore = []
    Kg = np.zeros(G, dtype=np.int64)
    for c in range(NCORES):
        lo = c * SHARD
        m = (dst >= lo) & (dst < lo + SHARD)
        e_dst = dst[m] - lo
        e_src = src[m]
        d = deg[lo:lo + SHARD]
        order = np.argsort(-d, kind="stable")
        rank = np.empty(SHARD, dtype=np.int64)
        rank[order] = np.arange(SHARD)
        dsort = np.concatenate([d[order], np.zeros(PSHARD - SHARD, np.int64)])
        for g in range(G):
            Kg[g] = max(Kg[g], max(1, dsort[g * 128:(g + 1) * 128].max()))
        percore.append(dict(order=order, rank=rank, e_dst=e_dst, e_src=e_src))

    pos1 = np.empty(N, dtype=np.int64)  # sorted-space padded id
    for c in range(NCORES):
        pos1[c * SHARD:(c + 1) * SHARD] = c * PSHARD + percore[c]["rank"]

    SK = int(Kg.sum())
    cums = np.concatenate([[0], np.cumsum(Kg)])
    for c in range(NCORES):
        pc = percore[c]
        sent_row = c * PSHARD + PSHARD - 1
        r = pc["rank"][pc["e_dst"]]
        # self-loop edges first within each dst -> they land in slot 0
        not_self = (pc["e_src"] != pc["e_dst"] + c * SHARD).astype(np.int64)
        ordr = np.lexsort((not_self, r))
        r_s = r[ordr]
        kpos = np.arange(len(r_s)) - np.searchsorted(r_s, r_s)
        Kcap = int(Kg.max())
        slots0 = np.full((PSHARD, Kcap), sent_row, dtype=np.int64)
        slots0[r_s, kpos] = pad_id[pc["e_src"][ordr]]
        slots1 = np.full((PSHARD, Kcap), sent_row, dtype=np.int64)
        slots1[r_s, kpos] = pos1[pc["e_src"][ordr]]
        # pack per-group [128, Kg[g]] -> [128, SK] (element offsets)
        offs0 = np.zeros((128, SK), np.int32)
        offs1 = np.zeros((128, SK), np.int32)
        for g in range(G):
            offs0[:, cums[g]:cums[g + 1]] = slots0[g * 128:(g + 1) * 128, :Kg[g]]
            offs1[:, cums[g]:cums[g + 1]] = slots1[g * 128:(g + 1) * 128, :Kg[g]]
        pc["offs0"] = offs0
        pc["offs1"] = offs1
    return percore, Kg.astype(int), cums.astype(int)


def _prep_weights(inputs):
    X = np.asarray(inputs["X"], np.float32)
    W0 = np.asarray(inputs["W0"], np.float32)
    al0 = np.asarray(inputs["al0"], np.float32)
    ar0 = np.asarray(inputs["ar0"], np.float32)
    b0 = np.asarray(inputs["b0"], np.float32)
    W1 = np.asarray(inputs["W1"], np.float32)
    al1 = np.asarray(inputs["al1"], np.float32)
    ar1 = np.asarray(inputs["ar1"], np.float32)
    b1 = np.asarray(inputs["b1"], np.float32)

    # W0 rearranged: [128, 12*140]; W0r[kp, k*140+j] = W0[k*128+kp, j]
    W0p = np.zeros((KDIM, D0), np.float32)
    W0p[:1433] = W0
    W0r = np.ascontiguousarray(
        W0p.reshape(12, 128, D0).transpose(1, 0, 2).reshape(128, 12 * D0))
    W1a = np.zeros((128, D1), np.float32)
    W1a[:128] = W1[:128]
    W1b = np.zeros((128, D1), np.float32)
    W1b[:12] = W1[128:140]
    bc = lambda v, w: np.broadcast_to(np.asarray(v, np.float32)[None, :],
                                      (128, w)).copy()
    wal1 = W1 @ al1
    war1 = W1 @ ar1
    ident = np.eye(128, dtype=np.float32)
    sent_mask = np.zeros((128, 1), np.float32)
    sent_mask[SHARD - (G - 1) * 128:, 0] = SENT   # partitions 84.. are pads
    com = dict(W0r=W0r, W1a=W1a, W1b=W1b, sent_mask=sent_mask,
               al0b=bc(al0, D0), ar0b=bc(ar0, D0), b0b=bc(b0, D0),
               wal1b=bc(wal1, D0), war1b=bc(war1, D0), b1b=bc(b1, D1),
               ident=ident)

    # X tiles per core: xt[n, kp, k*128+nf] = X[lo + n*128+nf, k*128+kp]
    xts = []
    for c in range(NCORES):
        lo = c * SHARD
        Xp = np.zeros((PSHARD, KDIM), np.float32)
        Xp[:SHARD, :1433] = X[lo:lo + SHARD, :]
        xt = Xp.reshape(G, 128, 12, 128).transpose(0, 3, 2, 1).reshape(G, 128, 12 * 128)
        xt = np.concatenate([xt, np.zeros((100 - G, 128, 12 * 128), np.float32)])
        xt = np.ascontiguousarray(
            xt.reshape(25, 4, 128, 1536).transpose(0, 2, 1, 3).reshape(25, 128, 4 * 1536))
        xts.append(xt)
    return com, xts


def _build(Kg, cums):
    import concourse.bass as bass
    import concourse.tile as tile
    from concourse import bacc, mybir
    dt = mybir.dt
    op = mybir.AluOpType
    act = mybir.ActivationFunctionType

    SK = int(sum(Kg))
    nc = bacc.Bacc("TRN2", target_bir_lowering=False, debug=False,
                   num_devices=NCORES)
    t_x = nc.dram_tensor("x_up", [25, 128, 4 * 12 * 128], dt.float32, kind="ExternalInput")
    t_w0 = nc.dram_tensor("w0r", [128, 12 * D0], dt.float32, kind="ExternalInput")
    t_w1a = nc.dram_tensor("w1a", [128, D1], dt.float32, kind="ExternalInput")
    t_w1b = nc.dram_tensor("w1b", [128, D1], dt.float32, kind="ExternalInput")
    t_al0 = nc.dram_tensor("al0b", [128, D0], dt.float32, kind="ExternalInput")
    t_ar0 = nc.dram_tensor("ar0b", [128, D0], dt.float32, kind="ExternalInput")
    t_b0 = nc.dram_tensor("b0b", [128, D0], dt.float32, kind="ExternalInput")
    t_wal1 = nc.dram_tensor("wal1b", [128, D0], dt.float32, kind="ExternalInput")
    t_war1 = nc.dram_tensor("war1b", [128, D0], dt.float32, kind="ExternalInput")
    t_b1 = nc.dram_tensor("b1b", [128, D1], dt.float32, kind="ExternalInput")
    t_id = nc.dram_tensor("ident", [128, 128], dt.float32, kind="ExternalInput")
    t_of0 = nc.dram_tensor("offs0", [128, SK], dt.int32, kind="ExternalInput")
    t_of1 = nc.dram_tensor("offs1", [128, SK], dt.int32, kind="ExternalInput")
    t_sm = nc.dram_tensor("sent_mask", [128, 1], dt.float32, kind="ExternalInput")
    t_out = nc.dram_tensor("out_buf", [PSHARD, D1], dt.float32, kind="ExternalOutput")

    with tile.TileContext(nc) as tc:
        with tc.tile_pool(name="const", bufs=1) as cpool, \
             tc.tile_pool(name="xload", bufs=2) as xpool, \
             tc.tile_pool(name="hex", bufs=3) as hexpool, \
             tc.tile_pool(name="gath", bufs=4) as gpool, \
             tc.tile_pool(name="work", bufs=3) as wpool, \
             tc.tile_pool(name="small", bufs=4) as spool, \
             tc.tile_pool(name="psum", bufs=2, space="PSUM") as ppool, \
             tc.tile_pool(name="dram", bufs=1, space="DRAM") as dpool:

            # constants
            w0_sb = cpool.tile([128, 12 * D0], dt.float32)
            nc.sync.dma_start(w0_sb[:], t_w0[:])
            w1a_sb = cpool.tile([128, D1], dt.float32)
            nc.sync.dma_start(w1a_sb[:], t_w1a[:])
            w1b_sb = cpool.tile([128, D1], dt.float32)
            nc.sync.dma_start(w1b_sb[:], t_w1b[:])
            al0_sb = cpool.tile([128, D0], dt.float32)
            nc.sync.dma_start(al0_sb[:], t_al0[:])
            ar0_sb = cpool.tile([128, D0], dt.float32)
            nc.sync.dma_start(ar0_sb[:], t_ar0[:])
            b0_sb = cpool.tile([128, D0], dt.float32)
            nc.sync.dma_start(b0_sb[:], t_b0[:])
            wal1_sb = cpool.tile([128, D0], dt.float32)
            nc.sync.dma_start(wal1_sb[:], t_wal1[:])
            war1_sb = cpool.tile([128, D0], dt.float32)
            nc.sync.dma_start(war1_sb[:], t_war1[:])
            b1_sb = cpool.tile([128, D1], dt.float32)
            nc.sync.dma_start(b1_sb[:], t_b1[:])
            id_sb = cpool.tile([128, 128], dt.float32)
            nc.sync.dma_start(id_sb[:], t_id[:])
            of0_sb = cpool.tile([128, SK], dt.int32)
            nc.sync.dma_start(of0_sb[:], t_of0[:])
            of1_sb = cpool.tile([128, SK], dt.int32)
            nc.sync.dma_start(of1_sb[:], t_of1[:])
            sm_sb = cpool.tile([128, 1], dt.float32)
            nc.sync.dma_start(sm_sb[:], t_sm[:])
            er1_all = cpool.tile([128, G], dt.float32)

            shard0 = dpool.tile([PSHARD, W0C], dt.float32)
            table0 = dpool.tile([NCORES * PSHARD, W0C], dt.float32, addr_space="Shared")
            shard1 = dpool.tile([PSHARD, W1C], dt.float32)
            table1 = dpool.tile([NCORES * PSHARD, W1C], dt.float32, addr_space="Shared")

            # ---- Phase M: h = X @ W0, pack [h, el, er] rows ----
            XB = 4                      # X tiles per DMA (amortize fixed cost)
            xts = {}
            for n in range(G):
                b, t = n // XB, n % XB
                if t == 0:
                    xtile = xpool.tile([128, 4 * 12 * 128], dt.float32, tag="xt")
                    xts[b] = xtile
                    nc.sync.dma_start(xtile[:], t_x[:][b])
                xt = xts[b][:, t * 1536:(t + 1) * 1536]
                ph = ppool.tile([128, D0], dt.float32, space="PSUM")
                for k in range(12):
                    nc.tensor.matmul(ph[:], xt[:, k * 128:(k + 1) * 128],
                                     w0_sb[:, k * D0:(k + 1) * D0],
                                     start=(k == 0), stop=(k == 11))
                hx = hexpool.tile([128, W0C], dt.float32, tag="hex0")
                nc.vector.tensor_copy(hx[:, 0:D0], ph[:])
                scr = wpool.tile([128, D0], dt.float32, tag="mscr")
                nc.vector.tensor_tensor(scr[:], ph[:], al0_sb[:], op=op.mult)
                nc.vector.tensor_reduce(hx[:, 140:141], scr[:],
                                        axis=mybir.AxisListType.X, op=op.add)
                nc.vector.tensor_tensor(scr[:], ph[:], ar0_sb[:], op=op.mult)
                nc.vector.tensor_reduce(hx[:, 141:142], scr[:],
                                        axis=mybir.AxisListType.X, op=op.add)
                nc.vector.memset(hx[:, 142:144], 0.0)
                if n == G - 1:
                    nc.vector.tensor_tensor(hx[:, 140:141], hx[:, 140:141],
                                            sm_sb[:], op=op.add)
                nc.sync.dma_start(shard0[:].rearrange("(g p) w -> g p w", p=128)[n],
                                  hx[:])

            nc.gpsimd.collective_compute(
                "AllGather", op.bypass, replica_groups=[list(range(NCORES))],
                ins=[shard0[:]], outs=[table0[:]])

            # ---- Phase E0 ----
            for g in range(G):
                K = int(Kg[g])
                gt = gpool.tile([128, K * W0C], dt.float32, tag="g0")
                gv = gt[:].rearrange("p (k w) -> p k w", w=W0C)
                for k in range(K):
                    nc.gpsimd.indirect_dma_start(
                        out=gv[:, k], out_offset=None, in_=table0[:],
                        in_offset=bass.IndirectOffsetOnAxis(
                            ap=of0_sb[:, cums[g] + k:cums[g] + k + 1], axis=0))
                # slot 0 is the self-loop -> its row IS the dst row; er = col 141
                ep = spool.tile([128, K], dt.float32, tag="ep0")
                nc.vector.tensor_scalar(ep[:], gv[:, :, 140], gv[:, 0, 141:142],
                                        None, op.add)
                ee = spool.tile([128, K], dt.float32, tag="ee0")
                nc.vector.scalar_tensor_tensor(
                    out=ee[:], in0=ep[:], scalar=0.2, in1=ep[:],
                    op0=op.mult, op1=op.max)
                ex = spool.tile([128, K], dt.float32, tag="ex0")
                dn = spool.tile([128, 1], dt.float32, tag="dn0")
                nc.scalar.activation(ex[:], ee[:], act.Exp, accum_out=dn[:])
                nc.vector.tensor_scalar_max(dn[:], dn[:], 1e-30)
                rv = spool.tile([128, 1], dt.float32, tag="rv0")
                nc.vector.reciprocal(rv[:], dn[:])
                acc = wpool.tile([128, D0], dt.float32, tag="acc0")
                nc.vector.tensor_scalar(acc[:], gv[:, 0, 0:D0], ex[:, 0:1], None,
                                        op.mult)
                for k in range(1, K):
                    nc.vector.scalar_tensor_tensor(
                        out=acc[:], in0=gv[:, k, 0:D0], scalar=ex[:, k:k + 1],
                        in1=acc[:], op0=op.mult, op1=op.add)
                h1 = wpool.tile([128, D0], dt.float32, tag="h1")
                nc.vector.scalar_tensor_tensor(
                    out=h1[:], in0=acc[:], scalar=rv[:], in1=b0_sb[:],
                    op0=op.mult, op1=op.add)
                nc.scalar.activation(h1[:], h1[:], act.Relu)
                # el1/er1
                hx1 = hexpool.tile([128, W1C], dt.float32, tag="hex1")
                scr1 = wpool.tile([128, D0], dt.float32, tag="escr")
                nc.vector.tensor_tensor(scr1[:], h1[:], wal1_sb[:], op=op.mult)
                nc.vector.tensor_reduce(hx1[:, 7:8], scr1[:],
                                        axis=mybir.AxisListType.X, op=op.add)
                nc.vector.tensor_tensor(scr1[:], h1[:], war1_sb[:], op=op.mult)
                nc.vector.tensor_reduce(hx1[:, 8:9], scr1[:],
                                        axis=mybir.AxisListType.X, op=op.add)
                nc.vector.tensor_copy(er1_all[:, g:g + 1], hx1[:, 8:9])
                # hp1 = h1 @ W1 via PE transpose
                pt1 = ppool.tile([128, 128], dt.float32, space="PSUM", tag="pt1")
                nc.tensor.transpose(pt1[:], h1[:, 0:128], id_sb[:])
                pt2 = ppool.tile([128, 128], dt.float32, space="PSUM", tag="pt2")
                nc.tensor.transpose(pt2[0:12, :], h1[:, 128:140], id_sb[:])
                t1s = wpool.tile([128, 128], dt.float32, tag="t1s")
                nc.vector.tensor_copy(t1s[:], pt1[:])
                t2s = wpool.tile([128, 128], dt.float32, tag="t2s")
                nc.vector.tensor_copy(t2s[0:12, :], pt2[0:12, :])
                php = ppool.tile([128, D1], dt.float32, space="PSUM", tag="php")
                nc.tensor.matmul(php[:], t1s[:], w1a_sb[:], start=True, stop=False)
                nc.tensor.matmul(php[:], t2s[0:12, :], w1b_sb[0:12, :],
                                 start=False, stop=True)
                nc.vector.tensor_copy(hx1[:, 0:D1], php[:])
                nc.vector.memset(hx1[:, 9:16], 0.0)
                if g == G - 1:
                    nc.vector.tensor_tensor(hx1[:, 7:8], hx1[:, 7:8],
                                            sm_sb[:], op=op.add)
                nc.sync.dma_start(shard1[:].rearrange("(g p) w -> g p w", p=128)[g],
                                  hx1[:])

            nc.gpsimd.collective_compute(
                "AllGather", op.bypass, replica_groups=[list(range(NCORES))],
                ins=[shard1[:]], outs=[table1[:]])

            # ---- Phase E1 ----
            for g in range(G):
                K = int(Kg[g])
                gt = gpool.tile([128, K * W1C], dt.float32, tag="g1")
                gv = gt[:].rearrange("p (k w) -> p k w", w=W1C)
                # slot 0 = self-loop: contiguous rows of our own shard (sorted space)
                nc.sync.dma_start(
                    gv[:, 0], shard1[:].rearrange("(g p) w -> g p w", p=128)[g])
                for k in range(1, K):
                    nc.gpsimd.indirect_dma_start(
                        out=gv[:, k], out_offset=None, in_=table1[:],
                        in_offset=bass.IndirectOffsetOnAxis(
                            ap=of1_sb[:, cums[g] + k:cums[g] + k + 1], axis=0))
                ep = spool.tile([128, K], dt.float32, tag="ep1")
                nc.vector.tensor_scalar(ep[:], gv[:, :, 7], er1_all[:, g:g + 1],
                                        None, op.add)
                ee = spool.tile([128, K], dt.float32, tag="ee1")
                nc.vector.scalar_tensor_tensor(
                    out=ee[:], in0=ep[:], scalar=0.2, in1=ep[:],
                    op0=op.mult, op1=op.max)
                ex = spool.tile([128, K], dt.float32, tag="ex1")
                dn = spool.tile([128, 1], dt.float32, tag="dn1")
                nc.scalar.activation(ex[:], ee[:], act.Exp, accum_out=dn[:])
                nc.vector.tensor_scalar_max(dn[:], dn[:], 1e-30)
                rv = spool.tile([128, 1], dt.float32, tag="rv1")
                nc.vector.reciprocal(rv[:], dn[:])
                acc = spool.tile([128, D1], dt.float32, tag="acc1")
                nc.vector.tensor_scalar(acc[:], gv[:, 0, 0:D1], ex[:, 0:1], None,
                                        op.mult)
                for k in range(1, K):
                    nc.vector.scalar_tensor_tensor(
                        out=acc[:], in0=gv[:, k, 0:D1], scalar=ex[:, k:k + 1],
                        in1=acc[:], op0=op.mult, op1=op.add)
                ot = spool.tile([128, D1], dt.float32, tag="ot")
                nc.vector.scalar_tensor_tensor(
                    out=ot[:], in0=acc[:], scalar=rv[:], in1=b1_sb[:],
                    op0=op.mult, op1=op.add)
                nc.scalar.activation(ot[:], ot[:], act.Relu)
                nc.sync.dma_start(t_out[:].rearrange("(g p) w -> g p w", p=128)[g],
                                  ot[:])
    nc.compile()
    return nc


def kernel(**inputs):
    percore, Kg, cums = _host_prep(inputs["src"], inputs["dst"])
    com, xts = _prep_weights(inputs)

    key = tuple(Kg)
    if key not in _CACHE:
        _CACHE[key] = _build(Kg, cums)
    nc = _CACHE[key]

    in_maps = []
    for c in range(NCORES):
        pc = percore[c]
        m = dict(x_up=xts[c], w0r=com["W0r"], w1a=com["W1a"], w1b=com["W1b"],
                 al0b=com["al0b"], ar0b=com["ar0b"], b0b=com["b0b"],
                 wal1b=com["wal1b"], war1b=com["war1b"], b1b=com["b1b"],
                 ident=com["ident"], offs0=pc["offs0"], offs1=pc["offs1"],
                 sent_mask=com["sent_mask"])
        in_maps.append(m)

    from concourse.bass_utils import run_bass_kernel_spmd
    global LAST_EXEC_NS
    res = run_bass_kernel_spmd(nc, in_maps, core_ids=list(range(NCORES)),
                               trace=TRACE)
    LAST_EXEC_NS = res.exec_time_ns
    out = np.zeros((N, D1), dtype=np.float32)
    for c in range(NCORES):
        ob = res.results[c]["out_buf"]
        out[c * SHARD + percore[c]["order"]] = ob[:SHARD]
    return out


# revision 35
# speedup vs baseline: 2286.0578x; 1.0024x over previous
"""Trainium2 Bass kernel for 2-layer single-head GAT (nn_GAT__80942953660642).

Strategy (8 NeuronCores, SPMD):
  - Nodes sharded contiguously: core c owns nodes [c*12500, (c+1)*12500).
  - Phase M: h = X_shard @ W0 on PE (host-pretransposed X tiles), el/er via
    DVE reduces; rows [h(140), el, er, pad] packed into a 144-f32 shard table.
  - AllGather the 7.2MB shard tables -> full 57.8MB node table per core.
  - Phase E0 (edge phase): per core, its dst nodes are degree-sorted into 98
    groups of 128 (one dst per partition). Each dst's incoming edges occupy
    padded slot columns; slot gathers use per-partition indirect DMA (128
    rows/call, int32 element offsets). Edge softmax without max-subtraction
    (numerically safe here); padding slots point at sentinel rows with
    el = -1e30 so exp() kills them. Weighted accumulation via fused DVE
    multiply-add over slot columns.
  - hp1 = h1 @ W1 (PE transpose + matmul), second 16-f32 table, AllGather,
    Phase E1 repeats the edge phase at width 7.
  - Host assembles the final [100000, 7] output (inverse degree-sort).
"""
import sys
sys.path.insert(0, "/opt/trn_rl_repo")
import numpy as np

N = 100000
NCORES = 8
SHARD = 12500
PSHARD = 12544          # 98 * 128
G = PSHARD // 128       # 98 groups
KDIM = 1536             # 1433 padded to 12*128
D0 = 140
D1 = 7
W0C = 144               # L0 table row: h(140), el(140), er(141), pad
W1C = 16                # L1 table row: hp1(7), el1(7), er1(8), pad
SENT = np.float32(-1e30)

_CACHE = {}
TRACE = False          # test harness sets this to capture an NTFF profile
LAST_EXEC_NS = None


def _host_prep(src, dst):
    src = np.asarray(src).astype(np.int64)
    dst = np.asarray(dst).astype(np.int64)
    deg = np.bincount(dst, minlength=N)
    nodes = np.arange(N, dtype=np.int64)
    pad_id = (nodes // SHARD) * PSHARD + (nodes % SHARD)  # original-order padded id

    percore = []
    Kg = np.zeros(G, dtype=np.int64)
    for c in range(NCORES):
        lo = c * SHARD
        m = (dst >= lo) & (dst < lo + SHARD)
        e_dst = dst[m] - lo
        e_src = src[m]
        d = deg[lo:lo + SHARD]
        order = np.argsort(-d, kind="stable")
        rank = np.empty(SHARD, dtype=np.int64)
        rank[order] = np.arange(SHARD)
        dsort = np.concatenate([d[order], np.zeros(PSHARD - SHARD, np.int64)])
        for g in range(G):
            Kg[g] = max(Kg[g], max(1, dsort[g * 128:(g + 1) * 128].max()))
        percore.append(dict(order=order, rank=rank, e_dst=e_dst, e_src=e_src))

    pos1 = np.empty(N, dtype=np.int64)  # sorted-space padded id
    for c in range(NCORES):
        pos1[c * SHARD:(c + 1) * SHARD] = c * PSHARD + percore[c]["rank"]

    SK = int(Kg.sum())
    cums = np.concatenate([[0], np.cumsum(Kg)])
    for c in range(NCORES):
        pc = percore[c]
        sent_row = c * PSHARD + PSHARD - 1
        r = pc["rank"][pc["e_dst"]]
        # self-loop edges first within each dst -> they land in slot 0
        not_self = (pc["e_src"] != pc["e_dst"] + c * SHARD).astype(np.int64)
        ordr = np.lexsort((not_self, r))
        r_s = r[ordr]
        kpos = np.arange(len(r_s)) - np.searchsorted(r_s, r_s)
        Kcap = int(Kg.max())
        slots0 = np.full((PSHARD, Kcap), sent_row, dtype=np.int64)
        slots0[r_s, kpos] = pad_id[pc["e_src"][ordr]]
        slots1 = np.full((PSHARD, Kcap), sent_row, dtype=np.int64)
        slots1[r_s, kpos] = pos1[pc["e_src"][ordr]]
        # pack per-group [128, Kg[g]] -> [128, SK] (element offsets)
        offs0 = np.zeros((128, SK), np.int32)
        offs1 = np.zeros((128, SK), np.int32)
        for g in range(G):
            offs0[:, cums[g]:cums[g + 1]] = slots0[g * 128:(g + 1) * 128, :Kg[g]]
            offs1[:, cums[g]:cums[g + 1]] = slots1[g * 128:(g + 1) * 128, :Kg[g]]
            # slot 0 is always own-core (self-loop or sentinel): make it
            # shard0-relative so the gather can run before the AllGather
            offs0[:, cums[g]] -= c * PSHARD
        pc["offs0"] = offs0
        pc["offs1"] = offs1
    return percore, Kg.astype(int), cums.astype(int)


def _prep_weights(inputs):
    X = np.asarray(inputs["X"], np.float32)
    W0 = np.asarray(inputs["W0"], np.float32)
    al0 = np.asarray(inputs["al0"], np.float32)
    ar0 = np.asarray(inputs["ar0"], np.float32)
    b0 = np.asarray(inputs["b0"], np.float32)
    W1 = np.asarray(inputs["W1"], np.float32)
    al1 = np.asarray(inputs["al1"], np.float32)
    ar1 = np.asarray(inputs["ar1"], np.float32)
    b1 = np.asarray(inputs["b1"], np.float32)

    # W0 rearranged: [128, 12*140]; W0r[kp, k*140+j] = W0[k*128+kp, j]
    W0p = np.zeros((KDIM, D0), np.float32)
    W0p[:1433] = W0
    W0r = np.ascontiguousarray(
        W0p.reshape(12, 128, D0).transpose(1, 0, 2).reshape(128, 12 * D0))
    W1a = np.zeros((128, D1), np.float32)
    W1a[:128] = W1[:128]
    W1b = np.zeros((128, D1), np.float32)
    W1b[:12] = W1[128:140]
    bc = lambda v, w: np.broadcast_to(np.asarray(v, np.float32)[None, :],
                                      (128, w)).copy()
    wal1 = W1 @ al1
    war1 = W1 @ ar1
    ident = np.eye(128, dtype=np.float32)
    sent_mask = np.zeros((128, 1), np.float32)
    sent_mask[SHARD - (G - 1) * 128:, 0] = SENT   # partitions 84.. are pads
    com = dict(W0r=W0r, W1a=W1a, W1b=W1b, sent_mask=sent_mask,
               al0b=bc(al0, D0), ar0b=bc(ar0, D0), b0b=bc(b0, D0),
               wal1b=bc(wal1, D0), war1b=bc(war1, D0), b1b=bc(b1, D1),
               ident=ident)

    # X tiles per core: xt[n, kp, k*128+nf] = X[lo + n*128+nf, k*128+kp]
    xts = []
    for c in range(NCORES):
        lo = c * SHARD
        Xp = np.zeros((PSHARD, KDIM), np.float32)
        Xp[:SHARD, :1433] = X[lo:lo + SHARD, :]
        xt = Xp.reshape(G, 128, 12, 128).transpose(0, 3, 2, 1).reshape(G, 128, 12 * 128)
        xt = np.concatenate([xt, np.zeros((100 - G, 128, 12 * 128), np.float32)])
        xt = np.ascontiguousarray(
            xt.reshape(25, 4, 128, 1536).transpose(0, 2, 1, 3).reshape(25, 128, 4 * 1536))
        xts.append(xt)
    return com, xts


def _build(Kg, cums):
    import concourse.bass as bass
    import concourse.tile as tile
    from concourse import bacc, mybir
    dt = mybir.dt
    op = mybir.AluOpType
    act = mybir.ActivationFunctionType

    SK = int(sum(Kg))
    nc = bacc.Bacc("TRN2", target_bir_lowering=False, debug=False,
                   num_devices=NCORES)
    t_x = nc.dram_tensor("x_up", [25, 128, 4 * 12 * 128], dt.float32, kind="ExternalInput")
    t_w0 = nc.dram_tensor("w0r", [128, 12 * D0], dt.float32, kind="ExternalInput")
    t_w1a = nc.dram_tensor("w1a", [128, D1], dt.float32, kind="ExternalInput")
    t_w1b = nc.dram_tensor("w1b", [128, D1], dt.float32, kind="ExternalInput")
    t_al0 = nc.dram_tensor("al0b", [128, D0], dt.float32, kind="ExternalInput")
    t_ar0 = nc.dram_tensor("ar0b", [128, D0], dt.float32, kind="ExternalInput")
    t_b0 = nc.dram_tensor("b0b", [128, D0], dt.float32, kind="ExternalInput")
    t_wal1 = nc.dram_tensor("wal1b", [128, D0], dt.float32, kind="ExternalInput")
    t_war1 = nc.dram_tensor("war1b", [128, D0], dt.float32, kind="ExternalInput")
    t_b1 = nc.dram_tensor("b1b", [128, D1], dt.float32, kind="ExternalInput")
    t_id = nc.dram_tensor("ident", [128, 128], dt.float32, kind="ExternalInput")
    t_of0 = nc.dram_tensor("offs0", [128, SK], dt.int32, kind="ExternalInput")
    t_of1 = nc.dram_tensor("offs1", [128, SK], dt.int32, kind="ExternalInput")
    t_sm = nc.dram_tensor("sent_mask", [128, 1], dt.float32, kind="ExternalInput")
    t_out = nc.dram_tensor("out_buf", [PSHARD, D1], dt.float32, kind="ExternalOutput")

    with tile.TileContext(nc) as tc:
        with tc.tile_pool(name="const", bufs=1) as cpool, \
             tc.tile_pool(name="xload", bufs=2) as xpool, \
             tc.tile_pool(name="hex", bufs=3) as hexpool, \
             tc.tile_pool(name="gath", bufs=4) as gpool, \
             tc.tile_pool(name="work", bufs=3) as wpool, \
             tc.tile_pool(name="small", bufs=4) as spool, \
             tc.tile_pool(name="psum", bufs=2, space="PSUM") as ppool, \
             tc.tile_pool(name="dram", bufs=1, space="DRAM") as dpool:

            # constants
            w0_sb = cpool.tile([128, 12 * D0], dt.float32)
            nc.sync.dma_start(w0_sb[:], t_w0[:])
            w1a_sb = cpool.tile([128, D1], dt.float32)
            nc.sync.dma_start(w1a_sb[:], t_w1a[:])
            w1b_sb = cpool.tile([128, D1], dt.float32)
            nc.sync.dma_start(w1b_sb[:], t_w1b[:])
            al0_sb = cpool.tile([128, D0], dt.float32)
            nc.sync.dma_start(al0_sb[:], t_al0[:])
            ar0_sb = cpool.tile([128, D0], dt.float32)
            nc.sync.dma_start(ar0_sb[:], t_ar0[:])
            b0_sb = cpool.tile([128, D0], dt.float32)
            nc.sync.dma_start(b0_sb[:], t_b0[:])
            wal1_sb = cpool.tile([128, D0], dt.float32)
            nc.sync.dma_start(wal1_sb[:], t_wal1[:])
            war1_sb = cpool.tile([128, D0], dt.float32)
            nc.sync.dma_start(war1_sb[:], t_war1[:])
            b1_sb = cpool.tile([128, D1], dt.float32)
            nc.sync.dma_start(b1_sb[:], t_b1[:])
            id_sb = cpool.tile([128, 128], dt.float32)
            nc.sync.dma_start(id_sb[:], t_id[:])
            of0_sb = cpool.tile([128, SK], dt.int32)
            nc.sync.dma_start(of0_sb[:], t_of0[:])
            of1_sb = cpool.tile([128, SK], dt.int32)
            nc.sync.dma_start(of1_sb[:], t_of1[:])
            sm_sb = cpool.tile([128, 1], dt.float32)
            nc.sync.dma_start(sm_sb[:], t_sm[:])
            er1_all = cpool.tile([128, G], dt.float32)

            shard0 = dpool.tile([PSHARD, W0C], dt.float32)
            table0 = dpool.tile([NCORES * PSHARD, W0C], dt.float32, addr_space="Shared")
            shard1 = dpool.tile([PSHARD, W1C], dt.float32)
            table1 = dpool.tile([NCORES * PSHARD, W1C], dt.float32, addr_space="Shared")

            # ---- Phase M: h = X @ W0, pack [h, el, er] rows ----
            XB = 4                      # X tiles per DMA (amortize fixed cost)
            xts = {}
            for n in range(G):
                b, t = n // XB, n % XB
                if t == 0:
                    xtile = xpool.tile([128, 4 * 12 * 128], dt.float32, tag="xt")
                    xts[b] = xtile
                    nc.sync.dma_start(xtile[:], t_x[:][b])
                xt = xts[b][:, t * 1536:(t + 1) * 1536]
                ph = ppool.tile([128, D0], dt.float32, space="PSUM")
                for k in range(12):
                    nc.tensor.matmul(ph[:], xt[:, k * 128:(k + 1) * 128],
                                     w0_sb[:, k * D0:(k + 1) * D0],
                                     start=(k == 0), stop=(k == 11))
                hx = hexpool.tile([128, W0C], dt.float32, tag="hex0")
                nc.vector.tensor_copy(hx[:, 0:D0], ph[:])
                scr = wpool.tile([128, D0], dt.float32, tag="mscr")
                nc.vector.tensor_tensor(scr[:], ph[:], al0_sb[:], op=op.mult)
                nc.vector.tensor_reduce(hx[:, 140:141], scr[:],
                                        axis=mybir.AxisListType.X, op=op.add)
                nc.vector.tensor_tensor(scr[:], ph[:], ar0_sb[:], op=op.mult)
                nc.vector.tensor_reduce(hx[:, 141:142], scr[:],
                                        axis=mybir.AxisListType.X, op=op.add)
                nc.vector.memset(hx[:, 142:144], 0.0)
                if n == G - 1:
                    nc.vector.tensor_tensor(hx[:, 140:141], hx[:, 140:141],
                                            sm_sb[:], op=op.add)
                nc.sync.dma_start(shard0[:].rearrange("(g p) w -> g p w", p=128)[n],
                                  hx[:])

            nc.gpsimd.collective_compute(
                "AllGather", op.bypass, replica_groups=[list(range(NCORES))],
                ins=[shard0[:]], outs=[table0[:]])

            # ---- Phase E0 ----
            for g in range(G):
                K = int(Kg[g])
                gt = gpool.tile([128, K * W0C], dt.float32, tag="g0")
                gv = gt[:].rearrange("p (k w) -> p k w", w=W0C)
                for k in range(K):
                    nc.gpsimd.indirect_dma_start(
                        out=gv[:, k], out_offset=None,
                        in_=shard0[:] if k == 0 else table0[:],
                        in_offset=bass.IndirectOffsetOnAxis(
                            ap=of0_sb[:, cums[g] + k:cums[g] + k + 1], axis=0))
                # slot 0 is the self-loop -> its row IS the dst row; er = col 141
                ep = spool.tile([128, K], dt.float32, tag="ep0")
                nc.vector.tensor_scalar(ep[:], gv[:, :, 140], gv[:, 0, 141:142],
                                        None, op.add)
                ee = spool.tile([128, K], dt.float32, tag="ee0")
                nc.vector.scalar_tensor_tensor(
                    out=ee[:], in0=ep[:], scalar=0.2, in1=ep[:],
                    op0=op.mult, op1=op.max)
                ex = spool.tile([128, K], dt.float32, tag="ex0")
                dn = spool.tile([128, 1], dt.float32, tag="dn0")
                nc.scalar.activation(ex[:], ee[:], act.Exp, accum_out=dn[:])
                nc.vector.tensor_scalar_max(dn[:], dn[:], 1e-30)
                rv = spool.tile([128, 1], dt.float32, tag="rv0")
                nc.vector.reciprocal(rv[:], dn[:])
                acc = wpool.tile([128, D0], dt.float32, tag="acc0")
                nc.vector.tensor_scalar(acc[:], gv[:, 0, 0:D0], ex[:, 0:1], None,
                                        op.mult)
                for k in range(1, K):
                    nc.vector.scalar_tensor_tensor(
                        out=acc[:], in0=gv[:, k, 0:D0], scalar=ex[:, k:k + 1],
                        in1=acc[:], op0=op.mult, op1=op.add)
                h1 = wpool.tile([128, D0], dt.float32, tag="h1")
                nc.vector.scalar_tensor_tensor(
                    out=h1[:], in0=acc[:], scalar=rv[:], in1=b0_sb[:],
                    op0=op.mult, op1=op.add)
                nc.scalar.activation(h1[:], h1[:], act.Relu)
                # el1/er1
                hx1 = hexpool.tile([128, W1C], dt.float32, tag="hex1")
                scr1 = wpool.tile([128, D0], dt.float32, tag="escr")
                nc.vector.tensor_tensor(scr1[:], h1[:], wal1_sb[:], op=op.mult)
                nc.vector.tensor_reduce(hx1[:, 7:8], scr1[:],
                                        axis=mybir.AxisListType.X, op=op.add)
                nc.vector.tensor_tensor(scr1[:], h1[:], war1_sb[:], op=op.mult)
                nc.vector.tensor_reduce(hx1[:, 8:9], scr1[:],
                                        axis=mybir.AxisListType.X, op=op.add)
                nc.vector.tensor_copy(er1_all[:, g:g + 1], hx1[:, 8:9])
                # hp1 = h1 @ W1 via PE transpose
                pt1 = ppool.tile([128, 128], dt.float32, space="PSUM", tag="pt1")
                nc.tensor.transpose(pt1[:], h1[:, 0:128], id_sb[:])
                pt2 = ppool.tile([128, 128], dt.float32, space="PSUM", tag="pt2")
                nc.tensor.transpose(pt2[0:12, :], h1[:, 128:140], id_sb[:])
                t1s = wpool.tile([128, 128], dt.float32, tag="t1s")
                nc.vector.tensor_copy(t1s[:], pt1[:])
                t2s = wpool.tile([128, 128], dt.float32, tag="t2s")
                nc.vector.tensor_copy(t2s[0:12, :], pt2[0:12, :])
                php = ppool.tile([128, D1], dt.float32, space="PSUM", tag="php")
                nc.tensor.matmul(php[:], t1s[:], w1a_sb[:], start=True, stop=False)
                nc.tensor.matmul(php[:], t2s[0:12, :], w1b_sb[0:12, :],
                                 start=False, stop=True)
                nc.vector.tensor_copy(hx1[:, 0:D1], php[:])
                nc.vector.memset(hx1[:, 9:16], 0.0)
                if g == G - 1:
                    nc.vector.tensor_tensor(hx1[:, 7:8], hx1[:, 7:8],
                                            sm_sb[:], op=op.add)
                nc.sync.dma_start(shard1[:].rearrange("(g p) w -> g p w", p=128)[g],
                                  hx1[:])

            nc.gpsimd.collective_compute(
                "AllGather", op.bypass, replica_groups=[list(range(NCORES))],
                ins=[shard1[:]], outs=[table1[:]])

            # ---- Phase E1 ----
            for g in range(G):
                K = int(Kg[g])
                gt = gpool.tile([128, K * W1C], dt.float32, tag="g1")
                gv = gt[:].rearrange("p (k w) -> p k w", w=W1C)
                # slot 0 = self-loop: contiguous rows of our own shard (sorted space)
                nc.sync.dma_start(
                    gv[:, 0], shard1[:].rearrange("(g p) w -> g p w", p=128)[g])
                for k in range(1, K):
                    nc.gpsimd.indirect_dma_start(
                        out=gv[:, k], out_offset=None, in_=table1[:],
                        in_offset=bass.IndirectOffsetOnAxis(
                            ap=of1_sb[:, cums[g] + k:cums[g] + k + 1], axis=0))
                ep = spool.tile([128, K], dt.float32, tag="ep1")
                nc.vector.tensor_scalar(ep[:], gv[:, :, 7], er1_all[:, g:g + 1],
                                        None, op.add)
                ee = spool.tile([128, K], dt.float32, tag="ee1")
                nc.vector.scalar_tensor_tensor(
                    out=ee[:], in0=ep[:], scalar=0.2, in1=ep[:],
                    op0=op.mult, op1=op.max)
                ex = spool.tile([128, K], dt.float32, tag="ex1")
                dn = spool.tile([128, 1], dt.float32, tag="dn1")
                nc.scalar.activation(ex[:], ee[:], act.Exp, accum_out=dn[:])
                nc.vector.tensor_scalar_max(dn[:], dn[:], 1e-30)
                rv = spool.tile([128, 1], dt.float32, tag="rv1")
                nc.vector.reciprocal(rv[:], dn[:])
                acc = spool.tile([128, D1], dt.float32, tag="acc1")
                nc.vector.tensor_scalar(acc[:], gv[:, 0, 0:D1], ex[:, 0:1], None,
                                        op.mult)
                for k in range(1, K):
                    nc.vector.scalar_tensor_tensor(
                        out=acc[:], in0=gv[:, k, 0:D1], scalar=ex[:, k:k + 1],
                        in1=acc[:], op0=op.mult, op1=op.add)
                ot = spool.tile([128, D1], dt.float32, tag="ot")
                nc.vector.scalar_tensor_tensor(
                    out=ot[:], in0=acc[:], scalar=rv[:], in1=b1_sb[:],
                    op0=op.mult, op1=op.add)
                nc.scalar.activation(ot[:], ot[:], act.Relu)
                nc.sync.dma_start(t_out[:].rearrange("(g p) w -> g p w", p=128)[g],
                                  ot[:])
    nc.compile()
    return nc


def kernel(**inputs):
    percore, Kg, cums = _host_prep(inputs["src"], inputs["dst"])
    com, xts = _prep_weights(inputs)

    key = tuple(Kg)
    if key not in _CACHE:
        _CACHE[key] = _build(Kg, cums)
    nc = _CACHE[key]

    in_maps = []
    for c in range(NCORES):
        pc = percore[c]
        m = dict(x_up=xts[c], w0r=com["W0r"], w1a=com["W1a"], w1b=com["W1b"],
                 al0b=com["al0b"], ar0b=com["ar0b"], b0b=com["b0b"],
                 wal1b=com["wal1b"], war1b=com["war1b"], b1b=com["b1b"],
                 ident=com["ident"], offs0=pc["offs0"], offs1=pc["offs1"],
                 sent_mask=com["sent_mask"])
        in_maps.append(m)

    from concourse.bass_utils import run_bass_kernel_spmd
    global LAST_EXEC_NS
    res = run_bass_kernel_spmd(nc, in_maps, core_ids=list(range(NCORES)),
                               trace=TRACE)
    LAST_EXEC_NS = res.exec_time_ns
    out = np.zeros((N, D1), dtype=np.float32)
    for c in range(NCORES):
        ob = res.results[c]["out_buf"]
        out[c * SHARD + percore[c]["order"]] = ob[:SHARD]
    return out
